# revision 20
# baseline (speedup 1.0000x reference)
"""Trainium2 Bass kernel for EnhancedCrossAttention.

Shapes (hardcoded): B=4, C=256, H=W=28, heads=8, head_dim=32.
Sharding: 8 cores = 4 batches x 2 head-groups (4 heads each core).
Each core computes its batch's QKV (its head-group's Q/K/V), attention for
4 heads, and a partial out-projection (contracting its 128 attention-output
channels). Host sums the two partials per batch, adds the folded bias,
folds the two spatial halves, and reshapes.

Host-side algebraic folds (all exact):
  - pos_emb enters only via the QKV matmul: b_eff = b_qkv + w_qkv @ pos
  - 1/sqrt(dh) folded into Q weights+bias
  - V bias contributes attn_out += b_v (softmax weights sum to 1), folded
    through w_out into a constant added on the host.

v2 design (vs v1): the v1 trace showed PE and ACT strictly alternating at a
~3.9us period per k-tile — the in-order PE queue was [scores_i,
AV_i(waits exp_i), scores_{i+1}], so scores_{i+1} could not run during
exp_i even though its inputs were ready. v2 software-pipelines: the scores
for k-tile group g+1 are emitted into the PE stream BEFORE the AV/den of
group g, so the PE works through them while ACT exps group g.

Hardware constraint (found by bisection): concurrent row-quadrant-packed
matmuls (tile_position=(32h, 0)) must NOT write the same PSUM bank — two
heads' score matmuls writing the same partitions of one bank crash the
exec unit (NRT_EXEC_UNIT_UNRECOVERABLE). So scores use ONE persistent
[128, 2048] tile where head h owns bank h (512 f32 cols), and chunks are
256 q-tokens: group g writes the 256-col half (g%2) of each head's bank.
The half alternation gives double-buffering within 4 banks; attnT (1) +
den (1) + a 2-buf utility pool (2) for QKV/V/out-proj = 8 banks exactly.
All matmul operands are bf16 (1 col/cycle; fp32 is 4) and are cast
host-side so no on-chip round-copy passes are needed. den's stationary
ones operand is [sz, 32], which lands the denominator broadcast across
each head's 32 partitions at identical stream cost, so normalization is
just reciprocal+multiply on DVE (no PE broadcast matmul). Q/K/V
production is deferred into the pipeline through the utility pool so the
exp stream starts ~2us in and QKV hides under early exps.
"""

import numpy as np

B, C, H, W = 4, 256, 28, 28
N = H * W            # 784
S = 2 * N            # 1568 tokens
NH = 8
DH = 32
GH = 4               # heads per group (per core)
GC = GH * DH         # 128 channels per group

CW = 256             # q-chunk width: 6 chunks of 256 + rump 32
NCH = 6
RQ0, RW = NCH * CW, S - NCH * CW   # 1536, 32
KTILES = [(i * 128, min(128, S - i * 128)) for i in range((S + 127) // 128)]
NKT = len(KTILES)    # 13 (12 full + one 32-row rump)

_cache = {}
LAST_RESULTS = None


def _build_nc(repeat=1):
    import concourse.mybir as mybir
    import concourse.tile as tile
    from concourse import bacc
    from contextlib import ExitStack

    f32 = mybir.dt.float32
    bf16 = mybir.dt.bfloat16
    Exp = mybir.ActivationFunctionType.Exp

    nc = bacc.Bacc("TRN2", target_bir_lowering=False, debug=False)

    x1_d = nc.dram_tensor("x1b", [C, N], bf16, kind="ExternalInput")
    x2_d = nc.dram_tensor("x2b", [C, N], bf16, kind="ExternalInput")
    wqk_d = nc.dram_tensor("wqkT", [C, 256], bf16, kind="ExternalInput")
    wv_d = nc.dram_tensor("wvT", [C, GC], bf16, kind="ExternalInput")
    wout_d = nc.dram_tensor("woutT", [GC, 256], bf16, kind="ExternalInput")
    bqk_d = nc.dram_tensor("bqk", [2, 128, 1], f32, kind="ExternalInput")
    y_d = nc.dram_tensor("y", [S, C], f32, kind="ExternalOutput")

    with tile.TileContext(nc) as tc:
      for _rep in range(repeat):
        ctx = ExitStack()
        pp = ctx.enter_context(tc.tile_pool(name="persist", bufs=1))
        sb = ctx.enter_context(tc.tile_pool(name="work", bufs=3))
        psb = ctx.enter_context(tc.tile_pool(name="pwork", bufs=3))
        stps = ctx.enter_context(tc.tile_pool(name="stps", bufs=1, space="PSUM"))
        avps = ctx.enter_context(tc.tile_pool(name="avps", bufs=1, space="PSUM"))
        dnps = ctx.enter_context(tc.tile_pool(name="dnps", bufs=1, space="PSUM"))
        utps = ctx.enter_context(tc.tile_pool(name="utps", bufs=2, space="PSUM"))

        xT = [pp.tile([128, S], bf16, name=f"xT{t}", tag=f"xT{t}") for t in range(2)]
        wqk_sb = [pp.tile([128, 256], bf16, name=f"wqk{t}", tag=f"wqk{t}") for t in range(2)]
        wv_sb = [pp.tile([128, GC], bf16, name=f"wv{t}", tag=f"wv{t}") for t in range(2)]
        wout_sb = pp.tile([128, 256], bf16, name="wout", tag="wout")
        bq_sb = pp.tile([128, 1], f32, name="bq", tag="bq")
        bk_sb = pp.tile([128, 1], f32, name="bk", tag="bk")
        QT = pp.tile([128, S], bf16, name="QT", tag="QT")
        KTt = pp.tile([128, S], bf16, name="KT", tag="KT")
        Vt = [pp.tile([128, GC], bf16, name=f"V{i}", tag=f"V{i}") for i in range(NKT)]
        ones_f = pp.tile([128, 32], f32, name="onesf", tag="onesf")
        ones_b = pp.tile([128, 32], bf16, name="onesb", tag="onesb")
        zc = pp.tile([128, 32], f32, name="zc", tag="zc")

        # the persistent 4-bank score tile: head h owns cols [512h, 512h+512)
        st = stps.tile([128, 2048], f32, name="st", tag="st")
        st_h = st[:].rearrange("p (h c) -> p h c", h=4)  # [128, 4, 512]

        # ones via exp(0): exact 1.0 and pre-loads the ACT exp table early
        nc.vector.memset(zc[:], 0.0)
        nc.scalar.activation(ones_f[:], zc[:], Exp)
        nc.vector.tensor_copy(ones_b[:], ones_f[:])

        # --- input DMA (bf16 direct); first 256 x-cols land first so
        # chunk 0's QKV/scores start early ---
        x1v = x1_d[:].rearrange("(t p) f -> t p f", p=128)
        x2v = x2_d[:].rearrange("(t p) f -> t p f", p=128)
        wqkv = wqk_d[:].rearrange("(t p) f -> t p f", p=128)
        wvv = wv_d[:].rearrange("(t p) f -> t p f", p=128)
        for t in range(2):
            nc.sync.dma_start(wqk_sb[t][:], wqkv[t])
            nc.sync.dma_start(xT[t][:, 0:CW], x1v[t][:, 0:CW])
            nc.sync.dma_start(xT[t][:, CW:N], x1v[t][:, CW:N])
            nc.sync.dma_start(xT[t][:, N:N + CW], x2v[t][:, 0:CW])
            nc.sync.dma_start(xT[t][:, N + CW:S], x2v[t][:, CW:N])
            nc.sync.dma_start(wv_sb[t][:], wvv[t])
        nc.sync.dma_start(bq_sb[:], bqk_d[0])
        nc.sync.dma_start(bk_sb[:], bqk_d[1])
        nc.sync.dma_start(wout_sb[:], wout_d[:])

        # ---- emit helpers ----
        def ut_tile(nm):
            # [128, 512] f32 = 1 bank; pool has 2 bufs
            return utps.tile([128, 512], f32, name=nm, tag="ut")

        def emit_qk(c0, c1, m, bias_t, out_t, nm):
            w = c1 - c0
            ps = ut_tile(nm)
            for t in range(2):
                nc.tensor.matmul(
                    ps[:, :w], wqk_sb[t][:, 128 * m:128 * m + 128], xT[t][:, c0:c1],
                    start=(t == 0), stop=(t == 1),
                )
            nc.vector.tensor_scalar_add(out_t[:, c0:c1], ps[:, :w], bias_t[:])

        def emit_v(i, nm):
            o, sz = KTILES[i]
            ps = ut_tile(nm)
            for t in range(2):
                nc.tensor.matmul(
                    ps[:sz, 0:GC], xT[t][:, o:o + sz], wv_sb[t][:],
                    start=(t == 0), stop=(t == 1),
                )
            nc.vector.tensor_copy(Vt[i][:sz, :], ps[:sz, 0:GC])

        def emit_scores(c0, w, i, half, off=0):
            # head h -> its own PSUM bank (concurrent row-packed matmuls
            # must not share a bank); halves alternate for double-buffering
            o, sz = KTILES[i]
            for h in range(4):
                base = 512 * h + CW * half + off
                nc.tensor.matmul(
                    st[:sz, base:base + w],
                    KTt[32 * h:32 * h + 32, o:o + sz],
                    QT[32 * h:32 * h + 32, c0:c0 + w],
                    start=True, stop=True,
                    tile_position=(32 * h, 0),
                )

        def emit_exp(w, sz, half, nm, off=0):
            P = psb.tile([128, 4 * CW], bf16, name=nm, tag="P")
            lo = CW * half + off
            stv = st_h[:sz, :, lo:lo + w]
            pv = P[:sz, :].rearrange("p (h c) -> p h c", h=4)[:, :, 0:w]
            nc.scalar.activation(pv, stv, Exp)
            return P

        def emit_avden(P, w, i, attnT_ps, den_ps, start, stop):
            o, sz = KTILES[i]
            for h in range(4):
                nc.tensor.matmul(
                    attnT_ps[32 * h:32 * h + 32, :w],
                    Vt[i][:sz, 32 * h:32 * h + 32],
                    P[:sz, CW * h:CW * h + w],
                    start=start, stop=stop, skip_group_check=True,
                    tile_position=(0, 32 * h),
                )
            for h in range(4):
                nc.tensor.matmul(
                    den_ps[32 * h:32 * h + 32, :w],
                    ones_b[:sz, :],
                    P[:sz, CW * h:CW * h + w],
                    start=start, stop=stop, skip_group_check=True,
                    tile_position=(0, 32 * h),
                )

        def emit_tail(c0, w, attnT_ps, den_ps, nm):
            # den is broadcast across each head's 32 partitions (ones lhsT
            # is [sz, 32]), so normalization is reciprocal + multiply only
            recip_f = sb.tile([128, CW], f32, name=f"rf{nm}", tag="recipf")
            nc.vector.reciprocal_approx_fast(recip_f[:, :w], den_ps[:, :w])
            attn_sb = sb.tile([128, CW], bf16, name=f"at{nm}", tag="attnsb")
            nc.vector.tensor_mul(attn_sb[:, :w], attnT_ps[:, :w], recip_f[:, :w])
            nsub = (w + 127) // 128
            for s4 in range(nsub):
                ssz = min(128, w - 128 * s4)
                off = 128 * s4
                yp = ut_tile(f"yp{nm}{s4}")
                nc.tensor.matmul(
                    yp[:ssz, 0:256], attn_sb[:, off:off + ssz], wout_sb[:],
                    start=True, stop=True,
                )
                ysb = sb.tile([128, 256], f32, name=f"ys{nm}{s4}", tag="ysb")
                nc.vector.tensor_copy(ysb[:ssz, :], yp[:ssz, 0:256])
                nc.sync.dma_start(y_d[c0 + off:c0 + off + ssz, :], ysb[:ssz, :])

        # ---- deferred production schedule ----
        # K chunk j covers score k-tiles 2j, 2j+1 -> keep 3-4 tiles ahead.
        # V k-tile j is consumed by group (0, j) -> produce at (0, j-2).
        # Q chunk c+1 is consumed from chunk c+1 -> produce mid-chunk c.
        pending = {}

        def defer(c, i, th):
            pending.setdefault((c, i), []).append(th)

        for j in range(2, NCH):
            defer(0, 2 * (j - 1) - 1, lambda j=j: emit_qk(
                CW * j, CW * (j + 1), 1, bk_sb, KTt, f"k{j}"))
        defer(0, 2 * NCH - 3, lambda: emit_qk(RQ0, S, 1, bk_sb, KTt, "kr"))
        for j in range(2, NKT):
            defer(0, j - 2, lambda j=j: emit_v(j, f"v{j}"))
        for c in range(NCH - 1):
            defer(c, 6, lambda c=c: emit_qk(
                CW * (c + 1), CW * (c + 2), 0, bq_sb, QT, f"q{c+1}"))
        defer(NCH - 1, 6, lambda: emit_qk(RQ0, S, 0, bq_sb, QT, "qr"))

        # ---- prologue ----
        emit_qk(0, CW, 0, bq_sb, QT, "q0")
        emit_qk(0, CW, 1, bk_sb, KTt, "k0")
        emit_qk(CW, 2 * CW, 1, bk_sb, KTt, "k1")
        emit_v(0, "v0")
        emit_v(1, "v1")

        # ---- main pipeline over NCH chunks x NKT ktiles ----
        groups = [(c, i) for c in range(NCH) for i in range(NKT)]
        emit_scores(0, CW, 0, 0)
        attnT_ps = den_ps = None
        tail_thunk = None
        for g, (c, i) in enumerate(groups):
            if i == 0:
                attnT_ps = avps.tile([128, CW], f32, name=f"attnT{c}", tag="attnT",
                                     padded_shape=[128, 512])
                den_ps = dnps.tile([128, CW], f32, name=f"den{c}", tag="den",
                                   padded_shape=[128, 512])
            o, sz = KTILES[i]
            P = emit_exp(CW, sz, g % 2, f"P{c}_{i}")
            # next group's scores run on PE during this exp
            if g + 1 < len(groups):
                nc2, ni = groups[g + 1]
                emit_scores(CW * nc2, CW, ni, (g + 1) % 2)
            # tail of the previous chunk must precede this chunk's first AV:
            # avps/dnps have one buf, so AV(c,0) waits on mul(c-1) via the
            # pool WAR edge — mul must already be in the DVE stream.
            if tail_thunk is not None:
                tail_thunk()
                tail_thunk = None
            emit_avden(P, CW, i, attnT_ps, den_ps, start=(i == 0), stop=(i == NKT - 1))
            for th in pending.get((c, i), ()):
                th()
            if i == NKT - 1:
                tail_thunk = (lambda c0=CW * c, a=attnT_ps, d=den_ps, c=c:
                              emit_tail(c0, CW, a, d, f"t{c}"))
        tail_thunk()

        # ---- rump chunk (q = 1536:1568, w=32) ----
        # 6 k-tiles packed per half of each head's bank (cols 512h +
        # 256*half + 32j), one wide strided exp per half; k-tile 12 in
        # half 0's cols 192:224 after its exp has freed them.
        attnT_ps = avps.tile([128, CW], f32, name="attnTr", tag="attnT",
                             padded_shape=[128, 512])
        den_ps = dnps.tile([128, CW], f32, name="denr", tag="den",
                           padded_shape=[128, 512])
        PRs = []
        for half in range(2):
            for j in range(6):
                i = 6 * half + j
                emit_scores(RQ0, RW, i, half, off=32 * j)
            PR = psb.tile([128, 4 * CW], bf16, name=f"PR{half}", tag="P")
            lo = CW * half
            stv = st_h[:, :, lo:lo + 192]
            pv = PR[:, :].rearrange("p (h c) -> p h c", h=4)[:, :, 0:192]
            nc.scalar.activation(pv, stv, Exp)
            PRs.append(PR)
        emit_scores(RQ0, RW, 12, 0, off=192)
        P2 = psb.tile([128, 4 * CW], bf16, name="P2", tag="P")
        o12, sz12 = KTILES[12]
        stv2 = st_h[:sz12, :, 192:224]
        pv2 = P2[:sz12, :].rearrange("p (h c) -> p h c", h=4)[:, :, 0:32]
        nc.scalar.activation(pv2, stv2, Exp)
        for half in range(2):
            for j in range(6):
                i = 6 * half + j
                o, sz = KTILES[i]
                for h in range(4):
                    nc.tensor.matmul(
                        attnT_ps[32 * h:32 * h + 32, :RW],
                        Vt[i][:sz, 32 * h:32 * h + 32],
                        PRs[half][:sz, CW * h + 32 * j:CW * h + 32 * j + 32],
                        start=(i == 0), stop=False, skip_group_check=True,
                        tile_position=(0, 32 * h),
                    )
                for h in range(4):
                    nc.tensor.matmul(
                        den_ps[32 * h:32 * h + 32, :RW],
                        ones_b[:sz, :],
                        PRs[half][:sz, CW * h + 32 * j:CW * h + 32 * j + 32],
                        start=(i == 0), stop=False, skip_group_check=True,
                        tile_position=(0, 32 * h),
                    )
        for h in range(4):
            nc.tensor.matmul(
                attnT_ps[32 * h:32 * h + 32, :RW],
                Vt[12][:sz12, 32 * h:32 * h + 32],
                P2[:sz12, CW * h:CW * h + 32],
                start=False, stop=True, skip_group_check=True,
                tile_position=(0, 32 * h),
            )
            nc.tensor.matmul(
                den_ps[32 * h:32 * h + 32, :RW],
                ones_b[:sz12, :],
                P2[:sz12, CW * h:CW * h + 32],
                start=False, stop=True, skip_group_check=True,
                tile_position=(0, 32 * h),
            )
        emit_tail(RQ0, RW, attnT_ps, den_ps, "tr")
        ctx.close()

    nc.compile()
    return nc


def prepare_in_maps(x1, x2, pos_emb, w_qkv, b_qkv, w_out, b_out):
    import ml_dtypes

    bf16 = ml_dtypes.bfloat16
    x1 = np.asarray(x1, dtype=np.float32)
    x2 = np.asarray(x2, dtype=np.float32)
    pos = np.asarray(pos_emb, dtype=np.float32).reshape(C)
    w_qkv = np.asarray(w_qkv, dtype=np.float32)
    b_qkv = np.asarray(b_qkv, dtype=np.float32)
    w_out = np.asarray(w_out, dtype=np.float32)
    b_out = np.asarray(b_out, dtype=np.float32)

    scale = 1.0 / np.sqrt(np.float32(DH))
    b_eff = b_qkv + w_qkv @ pos
    wq = w_qkv[0:C] * scale
    bq = b_eff[0:C] * scale
    wk = w_qkv[C:2 * C]
    bk = b_eff[C:2 * C]
    wv = w_qkv[2 * C:3 * C]
    bv = b_eff[2 * C:3 * C]

    in_maps = []
    for core in range(8):
        b = core // 2
        g = core % 2
        gsl = slice(GC * g, GC * (g + 1))
        wqkT = np.concatenate([wq[gsl], wk[gsl]], axis=0).T.copy()     # [C, 256]
        wvT = wv[gsl].T.copy()                                         # [C, GC]
        woutT = w_out[:, gsl].T.copy()                                 # [GC, 256]
        bqk = np.stack([bq[gsl], bk[gsl]])[:, :, None].copy()          # [2, 128, 1]
        in_maps.append({
            "x1b": np.ascontiguousarray(x1[b].reshape(C, N)).astype(bf16),
            "x2b": np.ascontiguousarray(x2[b].reshape(C, N)).astype(bf16),
            "wqkT": np.ascontiguousarray(wqkT).astype(bf16),
            "wvT": np.ascontiguousarray(wvT).astype(bf16),
            "woutT": np.ascontiguousarray(woutT).astype(bf16),
            "bqk": np.ascontiguousarray(bqk),
        })
    # out1+out2 folds two tokens, each carrying b_out and the V-bias term
    y_const = 2.0 * (b_out + w_out @ bv)  # [C]
    return in_maps, y_const


def get_nc(repeat=1):
    key = repeat
    if key not in _cache:
        _cache[key] = _build_nc(repeat)
    return _cache[key]


def assemble(per_core_y, y_const):
    out = np.empty((B, C, H, W), dtype=np.float32)
    for b in range(B):
        yb = per_core_y[2 * b] + per_core_y[2 * b + 1]                 # [S, C]
        yf = yb[:N] + yb[N:] + y_const[None, :]                        # [N, C]
        out[b] = yf.T.reshape(C, H, W)
    return out


def kernel(x1, x2, pos_emb, w_qkv, b_qkv, w_out, b_out):
    global LAST_RESULTS
    from concourse.bass_utils import run_bass_kernel_spmd

    in_maps, y_const = prepare_in_maps(x1, x2, pos_emb, w_qkv, b_qkv, w_out, b_out)
    nc = get_nc()
    res = run_bass_kernel_spmd(nc, in_maps, core_ids=list(range(8)))
    LAST_RESULTS = res
    return assemble([res.results[c]["y"] for c in range(8)], y_const)


# revision 24
# speedup vs baseline: 15712.4640x; 15712.4640x over previous
"""Trainium2 Bass kernel for EnhancedCrossAttention.

Shapes (hardcoded): B=4, C=256, H=W=28, heads=8, head_dim=32.
Sharding: 8 cores = 4 batches x 2 head-groups (4 heads each core).
Each core computes its batch's QKV (its head-group's Q/K/V), attention for
4 heads, and a partial out-projection (contracting its 128 attention-output
channels). Host sums the two partials per batch, adds the folded bias,
folds the two spatial halves, and reshapes.

Host-side algebraic folds (all exact):
  - pos_emb enters only via the QKV matmul: b_eff = b_qkv + w_qkv @ pos
  - 1/sqrt(dh) folded into Q weights+bias
  - V bias contributes attn_out += b_v (softmax weights sum to 1), folded
    through w_out into a constant added on the host.

v2 design (vs v1): the v1 trace showed PE and ACT strictly alternating at a
~3.9us period per k-tile — the in-order PE queue was [scores_i,
AV_i(waits exp_i), scores_{i+1}], so scores_{i+1} could not run during
exp_i even though its inputs were ready. v2 software-pipelines: the scores
for k-tile group g+1 are emitted into the PE stream BEFORE the AV/den of
group g, so the PE works through them while ACT exps group g.

Hardware constraint (found by bisection): concurrent row-quadrant-packed
matmuls (tile_position=(32h, 0)) must NOT write the same PSUM bank — two
heads' score matmuls writing the same partitions of one bank crash the
exec unit (NRT_EXEC_UNIT_UNRECOVERABLE). So scores use ONE persistent
[128, 2048] tile where head h owns bank h (512 f32 cols), and chunks are
256 q-tokens: group g writes the 256-col half (g%2) of each head's bank.
The half alternation gives double-buffering within 4 banks; attnT (1) +
den (1) + a 2-buf utility pool (2) for QKV/V/out-proj = 8 banks exactly.
All matmul operands are bf16 (1 col/cycle; fp32 is 4) and are cast
host-side so no on-chip round-copy passes are needed. den's stationary
ones operand is [sz, 32], which lands the denominator broadcast across
each head's 32 partitions at identical stream cost, so normalization is
just reciprocal+multiply on DVE (no PE broadcast matmul). Q/K/V
production is deferred into the pipeline through the utility pool so the
exp stream starts ~2us in and QKV hides under early exps.
"""

import numpy as np

B, C, H, W = 4, 256, 28, 28
N = H * W            # 784
S = 2 * N            # 1568 tokens
NH = 8
DH = 32
GH = 4               # heads per group (per core)
GC = GH * DH         # 128 channels per group

CW = 224             # q-chunk width: 7 chunks of 224, no q-rump (7*224=1568)
NCH = 7
KTILES = [(i * 128, min(128, S - i * 128)) for i in range((S + 127) // 128)]
NKT = len(KTILES)    # 13 (12 full + one 32-row k-rump)

_cache = {}
LAST_RESULTS = None
LAST_IN_MAPS = None


def _build_nc(repeat=1):
    import concourse.mybir as mybir
    import concourse.tile as tile
    from concourse import bacc
    from contextlib import ExitStack

    f32 = mybir.dt.float32
    bf16 = mybir.dt.bfloat16
    Exp = mybir.ActivationFunctionType.Exp

    nc = bacc.Bacc("TRN2", target_bir_lowering=False, debug=False)

    x1_d = nc.dram_tensor("x1b", [C, N], bf16, kind="ExternalInput")
    x2_d = nc.dram_tensor("x2b", [C, N], bf16, kind="ExternalInput")
    wqk_d = nc.dram_tensor("wqkT", [C, 256], bf16, kind="ExternalInput")
    wv_d = nc.dram_tensor("wvT", [C, GC], bf16, kind="ExternalInput")
    wout_d = nc.dram_tensor("woutT", [GC, 256], bf16, kind="ExternalInput")
    bqk_d = nc.dram_tensor("bqk", [2, 128, 1], f32, kind="ExternalInput")
    y_d = nc.dram_tensor("y", [S, C], f32, kind="ExternalOutput")

    with tile.TileContext(nc) as tc:
      for _rep in range(repeat):
        ctx = ExitStack()
        pp = ctx.enter_context(tc.tile_pool(name="persist", bufs=1))
        sb = ctx.enter_context(tc.tile_pool(name="work", bufs=3))
        psb = ctx.enter_context(tc.tile_pool(name="pwork", bufs=3))
        stps = ctx.enter_context(tc.tile_pool(name="stps", bufs=1, space="PSUM"))
        avps = ctx.enter_context(tc.tile_pool(name="avps", bufs=1, space="PSUM"))
        dnps = ctx.enter_context(tc.tile_pool(name="dnps", bufs=1, space="PSUM"))
        utps = ctx.enter_context(tc.tile_pool(name="utps", bufs=2, space="PSUM"))

        xT = [pp.tile([128, S], bf16, name=f"xT{t}", tag=f"xT{t}") for t in range(2)]
        wqk_sb = [pp.tile([128, 256], bf16, name=f"wqk{t}", tag=f"wqk{t}") for t in range(2)]
        wv_sb = [pp.tile([128, GC], bf16, name=f"wv{t}", tag=f"wv{t}") for t in range(2)]
        wout_sb = pp.tile([128, 256], bf16, name="wout", tag="wout")
        bq_sb = pp.tile([128, 1], f32, name="bq", tag="bq")
        bk_sb = pp.tile([128, 1], f32, name="bk", tag="bk")
        QT = pp.tile([128, S], bf16, name="QT", tag="QT")
        KTt = pp.tile([128, S], bf16, name="KT", tag="KT")
        Vt = [pp.tile([128, GC], bf16, name=f"V{i}", tag=f"V{i}") for i in range(NKT)]
        ones_f = pp.tile([128, 32], f32, name="onesf", tag="onesf")
        ones_b = pp.tile([128, 32], bf16, name="onesb", tag="onesb")
        zc = pp.tile([128, 32], f32, name="zc", tag="zc")

        # the persistent 4-bank score tile: head h owns cols [512h, 512h+512)
        st = stps.tile([128, 2048], f32, name="st", tag="st")
        st_h = st[:].rearrange("p (h c) -> p h c", h=4)  # [128, 4, 512]

        # ones via exp(0): exact 1.0 and pre-loads the ACT exp table early
        nc.vector.memset(zc[:], 0.0)
        nc.scalar.activation(ones_f[:], zc[:], Exp)
        nc.vector.tensor_copy(ones_b[:], ones_f[:])

        # --- input DMA (bf16 direct); first 256 x-cols land first so
        # chunk 0's QKV/scores start early ---
        x1v = x1_d[:].rearrange("(t p) f -> t p f", p=128)
        x2v = x2_d[:].rearrange("(t p) f -> t p f", p=128)
        wqkv = wqk_d[:].rearrange("(t p) f -> t p f", p=128)
        wvv = wv_d[:].rearrange("(t p) f -> t p f", p=128)
        for t in range(2):
            nc.sync.dma_start(wqk_sb[t][:], wqkv[t])
            nc.sync.dma_start(xT[t][:, 0:CW], x1v[t][:, 0:CW])
            nc.sync.dma_start(xT[t][:, CW:N], x1v[t][:, CW:N])
            nc.sync.dma_start(xT[t][:, N:N + CW], x2v[t][:, 0:CW])
            nc.sync.dma_start(xT[t][:, N + CW:S], x2v[t][:, CW:N])
            nc.sync.dma_start(wv_sb[t][:], wvv[t])
        nc.sync.dma_start(bq_sb[:], bqk_d[0])
        nc.sync.dma_start(bk_sb[:], bqk_d[1])
        nc.sync.dma_start(wout_sb[:], wout_d[:])

        # ---- emit helpers ----
        def ut_tile(nm):
            # [128, 512] f32 = 1 bank; pool has 2 bufs
            return utps.tile([128, 512], f32, name=nm, tag="ut")

        def emit_qk(c0, c1, m, bias_t, out_t, nm):
            w = c1 - c0
            ps = ut_tile(nm)
            for t in range(2):
                nc.tensor.matmul(
                    ps[:, :w], wqk_sb[t][:, 128 * m:128 * m + 128], xT[t][:, c0:c1],
                    start=(t == 0), stop=(t == 1),
                )
            nc.vector.tensor_scalar_add(out_t[:, c0:c1], ps[:, :w], bias_t[:])

        def emit_v(i, nm):
            o, sz = KTILES[i]
            ps = ut_tile(nm)
            for t in range(2):
                nc.tensor.matmul(
                    ps[:sz, 0:GC], xT[t][:, o:o + sz], wv_sb[t][:],
                    start=(t == 0), stop=(t == 1),
                )
            nc.vector.tensor_copy(Vt[i][:sz, :], ps[:sz, 0:GC])

        def emit_scores(c0, w, i, half, off=0):
            # head h -> its own PSUM bank (concurrent row-packed matmuls
            # must not share a bank); halves alternate for double-buffering
            o, sz = KTILES[i]
            for h in range(4):
                base = 512 * h + 256 * half + off
                nc.tensor.matmul(
                    st[:sz, base:base + w],
                    KTt[32 * h:32 * h + 32, o:o + sz],
                    QT[32 * h:32 * h + 32, c0:c0 + w],
                    start=True, stop=True,
                    tile_position=(32 * h, 0),
                )

        def emit_exp(w, sz, half, P, jj, nm, off=0):
            # P is a pair tile [128, 2048]: j-half jj, head h at 1024*jj+256*h
            lo = 256 * half + off
            stv = st_h[:sz, :, lo:lo + w]
            pv = P[:sz, :].rearrange("p (j h c) -> p j h c", j=2, h=4)[:, jj, :, 0:w]
            nc.scalar.activation(pv, stv, Exp)

        def emit_av(P, jj, w, i, attnT_ps, start, stop):
            o, sz = KTILES[i]
            for h in range(4):
                nc.tensor.matmul(
                    attnT_ps[32 * h:32 * h + 32, :w],
                    Vt[i][:sz, 32 * h:32 * h + 32],
                    P[:sz, 1024 * jj + 256 * h:1024 * jj + 256 * h + w],
                    start=start, stop=stop, skip_group_check=True,
                    tile_position=(0, 32 * h),
                )

        def emit_den_single(P, jj, w, i, den_ps, start, stop):
            o, sz = KTILES[i]
            for h in range(4):
                nc.tensor.matmul(
                    den_ps[32 * h:32 * h + 32, 0:w],
                    ones_b[:sz, :],
                    P[:sz, 1024 * jj + 256 * h:1024 * jj + 256 * h + w],
                    start=start, stop=stop, skip_group_check=True,
                    tile_position=(0, 32 * h),
                )

        def emit_tail(c0, w, attnT_ps, den_ps, nm):
            # den is broadcast across each head's 32 partitions (ones lhsT
            # is [sz, 32]), so normalization is reciprocal + multiply only.
            recip_f = sb.tile([128, CW], f32, name=f"rf{nm}", tag="recipf")
            nc.vector.reciprocal_approx_fast(recip_f[:, :w], den_ps[:, :w])
            attn_sb = sb.tile([128, CW], bf16, name=f"at{nm}", tag="attnsb")
            nc.vector.tensor_mul(attn_sb[:, :w], attnT_ps[:, :w], recip_f[:, :w])
            nsub = (w + 127) // 128
            for s4 in range(nsub):
                ssz = min(128, w - 128 * s4)
                off = 128 * s4
                yp = ut_tile(f"yp{nm}{s4}")
                nc.tensor.matmul(
                    yp[:ssz, 0:256], attn_sb[:, off:off + ssz], wout_sb[:],
                    start=True, stop=True,
                )
                ysb = sb.tile([128, 256], f32, name=f"ys{nm}{s4}", tag="ysb")
                nc.vector.tensor_copy(ysb[:ssz, :], yp[:ssz, 0:256])
                nc.sync.dma_start(y_d[c0 + off:c0 + off + ssz, :], ysb[:ssz, :])

        # ---- deferred production schedule ----
        # K chunk j covers score k-tiles 2j, 2j+1 -> keep 3-4 tiles ahead.
        # V k-tile j is consumed by group (0, j) -> produce at (0, j-2).
        # Q chunk c+1 is consumed from chunk c+1 -> produce mid-chunk c.
        pending = {}

        def defer(c, i, th):
            pending.setdefault((c, i), []).append(th)

        # K chunk j covers tokens [224j, 224j+224); first score k-tile
        # touching it is floor(224j/128) — emit 2 k-tiles ahead.
        for j in range(2, NCH):
            defer(0, max(0, 224 * j // 128 - 2), lambda j=j: emit_qk(
                CW * j, CW * (j + 1), 1, bk_sb, KTt, f"k{j}"))
        for j in range(2, NKT):
            defer(0, j - 2, lambda j=j: emit_v(j, f"v{j}"))
        for c in range(NCH - 1):
            defer(c, 6, lambda c=c: emit_qk(
                CW * (c + 1), CW * (c + 2), 0, bq_sb, QT, f"q{c+1}"))

        # ---- prologue ----
        emit_qk(0, CW, 0, bq_sb, QT, "q0")
        emit_qk(0, CW, 1, bk_sb, KTt, "k0")
        emit_qk(CW, 2 * CW, 1, bk_sb, KTt, "k1")
        emit_v(0, "v0")
        emit_v(1, "v1")

        # ---- main pipeline over NCH chunks x NKT ktiles ----
        groups = [(c, i) for c in range(NCH) for i in range(NKT)]
        emit_scores(0, CW, 0, 0)
        attnT_ps = den_ps = None
        P_pair = None
        tail_thunk = None
        for g, (c, i) in enumerate(groups):
            if i == 0:
                attnT_ps = avps.tile([128, CW], f32, name=f"attnT{c}", tag="attnT",
                                     padded_shape=[128, 512])
                den_ps = dnps.tile([128, 2 * CW], f32, name=f"den{c}", tag="den",
                                   padded_shape=[128, 512])
            o, sz = KTILES[i]
            jj = i % 2
            if jj == 0:
                P_pair = psb.tile([128, 2048], bf16, name=f"P{c}_{i}", tag="P")
            emit_exp(CW, sz, g % 2, P_pair, jj, f"e{c}_{i}")
            # next group's scores run on PE during this exp
            if g + 1 < len(groups):
                nc2, ni = groups[g + 1]
                emit_scores(CW * nc2, CW, ni, (g + 1) % 2)
            # tail of the previous chunk must precede this chunk's first AV:
            # avps/dnps have one buf, so AV(c,0) waits on mul(c-1) via the
            # pool WAR edge — mul must already be in the DVE stream.
            if tail_thunk is not None:
                tail_thunk()
                tail_thunk = None
            emit_av(P_pair, jj, CW, i, attnT_ps, start=(i == 0), stop=(i == NKT - 1))
            emit_den_single(P_pair, jj, CW, i, den_ps, start=(i == 0),
                            stop=(i == NKT - 1))
            for th in pending.get((c, i), ()):
                th()
            if i == NKT - 1:
                tail_thunk = (lambda c0=CW * c, a=attnT_ps, d=den_ps, c=c:
                              emit_tail(c0, CW, a, d, f"t{c}"))
        tail_thunk()

        ctx.close()

    nc.compile()
    return nc


def prepare_in_maps(x1, x2, pos_emb, w_qkv, b_qkv, w_out, b_out):
    import ml_dtypes

    bf16 = ml_dtypes.bfloat16
    x1 = np.asarray(x1, dtype=np.float32)
    x2 = np.asarray(x2, dtype=np.float32)
    pos = np.asarray(pos_emb, dtype=np.float32).reshape(C)
    w_qkv = np.asarray(w_qkv, dtype=np.float32)
    b_qkv = np.asarray(b_qkv, dtype=np.float32)
    w_out = np.asarray(w_out, dtype=np.float32)
    b_out = np.asarray(b_out, dtype=np.float32)

    scale = 1.0 / np.sqrt(np.float32(DH))
    b_eff = b_qkv + w_qkv @ pos
    wq = w_qkv[0:C] * scale
    bq = b_eff[0:C] * scale
    wk = w_qkv[C:2 * C]
    bk = b_eff[C:2 * C]
    wv = w_qkv[2 * C:3 * C]
    bv = b_eff[2 * C:3 * C]

    in_maps = []
    for core in range(8):
        b = core // 2
        g = core % 2
        gsl = slice(GC * g, GC * (g + 1))
        wqkT = np.concatenate([wq[gsl], wk[gsl]], axis=0).T.copy()     # [C, 256]
        wvT = wv[gsl].T.copy()                                         # [C, GC]
        woutT = w_out[:, gsl].T.copy()                                 # [GC, 256]
        bqk = np.stack([bq[gsl], bk[gsl]])[:, :, None].copy()          # [2, 128, 1]
        in_maps.append({
            "x1b": np.ascontiguousarray(x1[b].reshape(C, N)).astype(bf16),
            "x2b": np.ascontiguousarray(x2[b].reshape(C, N)).astype(bf16),
            "wqkT": np.ascontiguousarray(wqkT).astype(bf16),
            "wvT": np.ascontiguousarray(wvT).astype(bf16),
            "woutT": np.ascontiguousarray(woutT).astype(bf16),
            "bqk": np.ascontiguousarray(bqk),
        })
    # out1+out2 folds two tokens, each carrying b_out and the V-bias term
    y_const = 2.0 * (b_out + w_out @ bv)  # [C]
    return in_maps, y_const


def get_nc(repeat=1):
    key = repeat
    if key not in _cache:
        _cache[key] = _build_nc(repeat)
    return _cache[key]


def assemble(per_core_y, y_const):
    out = np.empty((B, C, H, W), dtype=np.float32)
    for b in range(B):
        yb = per_core_y[2 * b] + per_core_y[2 * b + 1]                 # [S, C]
        yf = yb[:N] + yb[N:] + y_const[None, :]                        # [N, C]
        out[b] = yf.T.reshape(C, H, W)
    return out


def kernel(x1, x2, pos_emb, w_qkv, b_qkv, w_out, b_out):
    global LAST_RESULTS, LAST_IN_MAPS
    from concourse.bass_utils import run_bass_kernel_spmd

    in_maps, y_const = prepare_in_maps(x1, x2, pos_emb, w_qkv, b_qkv, w_out, b_out)
    LAST_IN_MAPS = in_maps
    nc = get_nc()
    res = run_bass_kernel_spmd(nc, in_maps, core_ids=list(range(8)))
    LAST_RESULTS = res
    return assemble([res.results[c]["y"] for c in range(8)], y_const)


# revision 28
# speedup vs baseline: 15824.1259x; 1.0071x over previous
"""Trainium2 Bass kernel for EnhancedCrossAttention.

Shapes (hardcoded): B=4, C=256, H=W=28, heads=8, head_dim=32.
Sharding: 8 cores = 4 batches x 2 head-groups (4 heads each core).
Each core computes its batch's QKV (its head-group's Q/K/V), attention for
4 heads, and a partial out-projection (contracting its 128 attention-output
channels). Host sums the two partials per batch, adds the folded bias,
folds the two spatial halves, and reshapes.

Host-side algebraic folds (all exact):
  - pos_emb enters only via the QKV matmul: b_eff = b_qkv + w_qkv @ pos
  - 1/sqrt(dh) folded into Q weights+bias
  - V bias contributes attn_out += b_v (softmax weights sum to 1), folded
    through w_out into a constant added on the host.

v2 design (vs v1): the v1 trace showed PE and ACT strictly alternating at a
~3.9us period per k-tile — the in-order PE queue was [scores_i,
AV_i(waits exp_i), scores_{i+1}], so scores_{i+1} could not run during
exp_i even though its inputs were ready. v2 software-pipelines: the scores
for k-tile group g+1 are emitted into the PE stream BEFORE the AV/den of
group g, so the PE works through them while ACT exps group g.

Hardware constraint (found by bisection): concurrent row-quadrant-packed
matmuls (tile_position=(32h, 0)) must NOT write the same PSUM bank — two
heads' score matmuls writing the same partitions of one bank crash the
exec unit (NRT_EXEC_UNIT_UNRECOVERABLE). So scores use ONE persistent
[128, 2048] tile where head h owns bank h (512 f32 cols), and chunks are
256 q-tokens: group g writes the 256-col half (g%2) of each head's bank.
The half alternation gives double-buffering within 4 banks; attnT (1) +
den (1) + a 2-buf utility pool (2) for QKV/V/out-proj = 8 banks exactly.
All matmul operands are bf16 (1 col/cycle; fp32 is 4) and are cast
host-side so no on-chip round-copy passes are needed. den's stationary
ones operand is [sz, 32], which lands the denominator broadcast across
each head's 32 partitions at identical stream cost, so normalization is
just reciprocal+multiply on DVE (no PE broadcast matmul). Q/K/V
production is deferred into the pipeline through the utility pool so the
exp stream starts ~2us in and QKV hides under early exps.
"""

import numpy as np

B, C, H, W = 4, 256, 28, 28
N = H * W            # 784
S = 2 * N            # 1568 tokens
NH = 8
DH = 32
GH = 4               # heads per group (per core)
GC = GH * DH         # 128 channels per group

PW = 256             # head stride half-unit in the score tile
CHUNKS = [(0, 256), (256, 256), (512, 256), (768, 256), (1024, 256),
          (1280, 256), (1536, 32)]
NCH = len(CHUNKS)    # 7 chunks covering 1568 q-tokens exactly
KTILES = [(i * 128, min(128, S - i * 128)) for i in range((S + 127) // 128)]
NKT = len(KTILES)    # 13 (12 full + one 32-row k-rump)

_cache = {}
LAST_RESULTS = None
LAST_IN_MAPS = None


def _build_nc(repeat=1):
    import concourse.mybir as mybir
    import concourse.tile as tile
    from concourse import bacc
    from contextlib import ExitStack

    f32 = mybir.dt.float32
    bf16 = mybir.dt.bfloat16
    Exp = mybir.ActivationFunctionType.Exp

    nc = bacc.Bacc("TRN2", target_bir_lowering=False, debug=False)

    x1_d = nc.dram_tensor("x1b", [C, N], bf16, kind="ExternalInput")
    x2_d = nc.dram_tensor("x2b", [C, N], bf16, kind="ExternalInput")
    wqk_d = nc.dram_tensor("wqkT", [C, 256], bf16, kind="ExternalInput")
    wv_d = nc.dram_tensor("wvT", [C, GC], bf16, kind="ExternalInput")
    wout_d = nc.dram_tensor("woutT", [GC, 256], bf16, kind="ExternalInput")
    bqk_d = nc.dram_tensor("bqk", [2, 128, 1], f32, kind="ExternalInput")
    y_d = nc.dram_tensor("y", [S, C], f32, kind="ExternalOutput")

    with tile.TileContext(nc) as tc:
      for _rep in range(repeat):
        ctx = ExitStack()
        pp = ctx.enter_context(tc.tile_pool(name="persist", bufs=1))
        sb = ctx.enter_context(tc.tile_pool(name="work", bufs=3))
        psb = ctx.enter_context(tc.tile_pool(name="pwork", bufs=3))
        stps = ctx.enter_context(tc.tile_pool(name="stps", bufs=1, space="PSUM"))
        avps = ctx.enter_context(tc.tile_pool(name="avps", bufs=1, space="PSUM"))
        dnps = ctx.enter_context(tc.tile_pool(name="dnps", bufs=1, space="PSUM"))
        utps = ctx.enter_context(tc.tile_pool(name="utps", bufs=2, space="PSUM"))

        xT = [pp.tile([128, S], bf16, name=f"xT{t}", tag=f"xT{t}") for t in range(2)]
        wqk_sb = [pp.tile([128, 256], bf16, name=f"wqk{t}", tag=f"wqk{t}") for t in range(2)]
        wv_sb = [pp.tile([128, GC], bf16, name=f"wv{t}", tag=f"wv{t}") for t in range(2)]
        wout_sb = pp.tile([128, 256], bf16, name="wout", tag="wout")
        bq_sb = pp.tile([128, 1], f32, name="bq", tag="bq")
        bk_sb = pp.tile([128, 1], f32, name="bk", tag="bk")
        QT = pp.tile([128, S], bf16, name="QT", tag="QT")
        KTt = pp.tile([128, S], bf16, name="KT", tag="KT")
        Vt = [pp.tile([128, GC], bf16, name=f"V{i}", tag=f"V{i}") for i in range(NKT)]
        ones_f = pp.tile([128, 32], f32, name="onesf", tag="onesf")
        ones_b = pp.tile([128, 32], bf16, name="onesb", tag="onesb")
        zc = pp.tile([128, 32], f32, name="zc", tag="zc")

        # the persistent 4-bank score tile: head h owns bank h (512 cols;
        # two 256-col halves, consecutive groups alternate halves — matmul
        # dsts must not cross a PSUM bank boundary)
        st = stps.tile([128, 2048], f32, name="st", tag="st")
        st_h = st[:].rearrange("p (h c) -> p h c", h=4)  # [128, 4, 512]

        # ones via exp(0): exact 1.0 and pre-loads the ACT exp table early
        nc.vector.memset(zc[:], 0.0)
        nc.scalar.activation(ones_f[:], zc[:], Exp)
        nc.vector.tensor_copy(ones_b[:], ones_f[:])

        # --- input DMA (bf16 direct); first 320 x-cols land first so
        # chunk 0's QKV/scores start early ---
        x1v = x1_d[:].rearrange("(t p) f -> t p f", p=128)
        x2v = x2_d[:].rearrange("(t p) f -> t p f", p=128)
        wqkv = wqk_d[:].rearrange("(t p) f -> t p f", p=128)
        wvv = wv_d[:].rearrange("(t p) f -> t p f", p=128)
        for t in range(2):
            nc.sync.dma_start(wqk_sb[t][:], wqkv[t])
            nc.sync.dma_start(xT[t][:, 0:PW], x1v[t][:, 0:PW])
            nc.sync.dma_start(xT[t][:, PW:N], x1v[t][:, PW:N])
            nc.sync.dma_start(xT[t][:, N:N + PW], x2v[t][:, 0:PW])
            nc.sync.dma_start(xT[t][:, N + PW:S], x2v[t][:, PW:N])
            nc.sync.dma_start(wv_sb[t][:], wvv[t])
        nc.sync.dma_start(bq_sb[:], bqk_d[0])
        nc.sync.dma_start(bk_sb[:], bqk_d[1])
        nc.sync.dma_start(wout_sb[:], wout_d[:])

        # ---- emit helpers ----
        def ut_tile(nm):
            # [128, 512] f32 = 1 bank; pool has 2 bufs
            return utps.tile([128, 512], f32, name=nm, tag="ut")

        def emit_qk(c0, c1, m, bias_t, out_t, nm):
            w = c1 - c0
            ps = ut_tile(nm)
            for t in range(2):
                nc.tensor.matmul(
                    ps[:, :w], wqk_sb[t][:, 128 * m:128 * m + 128], xT[t][:, c0:c1],
                    start=(t == 0), stop=(t == 1),
                )
            nc.vector.tensor_scalar_add(out_t[:, c0:c1], ps[:, :w], bias_t[:])

        def emit_v(i, nm):
            o, sz = KTILES[i]
            ps = ut_tile(nm)
            for t in range(2):
                nc.tensor.matmul(
                    ps[:sz, 0:GC], xT[t][:, o:o + sz], wv_sb[t][:],
                    start=(t == 0), stop=(t == 1),
                )
            nc.vector.tensor_copy(Vt[i][:sz, :], ps[:sz, 0:GC])

        def emit_scores(c0, w, i, half, off=0):
            # head h -> its own PSUM bank (concurrent row-packed matmuls
            # must not share a bank); halves alternate for double-buffering
            o, sz = KTILES[i]
            for h in range(4):
                base = 512 * h + PW * half + off
                nc.tensor.matmul(
                    st[:sz, base:base + w],
                    KTt[32 * h:32 * h + 32, o:o + sz],
                    QT[32 * h:32 * h + 32, c0:c0 + w],
                    start=True, stop=True,
                    tile_position=(32 * h, 0),
                )

        def emit_exp(w, sz, half, P, nm, off=0):
            lo = PW * half + off
            stv = st_h[:sz, :, lo:lo + w]
            pv = P[:sz, :].rearrange("p (h c) -> p h c", h=4)[:, :, 0:w]
            nc.scalar.activation(pv, stv, Exp)

        def emit_av(P, w, i, attnT_ps, start, stop):
            o, sz = KTILES[i]
            for h in range(4):
                nc.tensor.matmul(
                    attnT_ps[32 * h:32 * h + 32, :w],
                    Vt[i][:sz, 32 * h:32 * h + 32],
                    P[:sz, PW * h:PW * h + w],
                    start=start, stop=stop, skip_group_check=True,
                    tile_position=(0, 32 * h),
                )

        def emit_den(P, w, i, den_ps, start, stop):
            o, sz = KTILES[i]
            for h in range(4):
                nc.tensor.matmul(
                    den_ps[32 * h:32 * h + 32, 0:w],
                    ones_b[:sz, :],
                    P[:sz, PW * h:PW * h + w],
                    start=start, stop=stop, skip_group_check=True,
                    tile_position=(0, 32 * h),
                )

        def emit_tail(c0, w, attnT_ps, den_ps, nm):
            # den is broadcast across each head's 32 partitions (ones lhsT
            # is [sz, 32]), so normalization is reciprocal + multiply only.
            recip_f = sb.tile([128, PW], f32, name=f"rf{nm}", tag="recipf")
            nc.vector.reciprocal_approx_fast(recip_f[:, :w], den_ps[:, :w])
            attn_sb = sb.tile([128, PW], bf16, name=f"at{nm}", tag="attnsb")
            nc.vector.tensor_mul(attn_sb[:, :w], attnT_ps[:, :w], recip_f[:, :w])
            nsub = (w + 127) // 128
            for s4 in range(nsub):
                ssz = min(128, w - 128 * s4)
                off = 128 * s4
                yp = ut_tile(f"yp{nm}{s4}")
                nc.tensor.matmul(
                    yp[:ssz, 0:256], attn_sb[:, off:off + ssz], wout_sb[:],
                    start=True, stop=True,
                )
                ysb = sb.tile([128, 256], f32, name=f"ys{nm}{s4}", tag="ysb")
                nc.vector.tensor_copy(ysb[:ssz, :], yp[:ssz, 0:256])
                nc.sync.dma_start(y_d[c0 + off:c0 + off + ssz, :], ysb[:ssz, :])

        # ---- deferred production schedule ----
        # K chunk j covers score k-tiles 2j, 2j+1 -> keep 3-4 tiles ahead.
        # V k-tile j is consumed by group (0, j) -> produce at (0, j-2).
        # Q chunk c+1 is consumed from chunk c+1 -> produce mid-chunk c.
        pending = {}

        def defer(c, i, th):
            pending.setdefault((c, i), []).append(th)

        # K chunk j covers tokens [c0, c0+w); first score k-tile touching
        # it is floor(c0/128) — emit 2 k-tiles ahead.
        for j in range(1, NCH):
            c0j, wj = CHUNKS[j]
            ii = max(0, c0j // 128 - 2)
            if j == 1:
                continue  # K1 in the prologue
            defer(0, ii, lambda c0j=c0j, wj=wj, j=j: emit_qk(
                c0j, c0j + wj, 1, bk_sb, KTt, f"k{j}"))
        for j in range(2, NKT):
            defer(0, j - 2, lambda j=j: emit_v(j, f"v{j}"))
        for c in range(NCH - 1):
            defer(c, 6, lambda c=c: emit_qk(
                CHUNKS[c + 1][0], CHUNKS[c + 1][0] + CHUNKS[c + 1][1],
                0, bq_sb, QT, f"q{c+1}"))

        # ---- prologue ----
        emit_qk(0, PW, 0, bq_sb, QT, "q0")
        emit_qk(0, PW, 1, bk_sb, KTt, "k0")
        emit_qk(PW, 2 * PW, 1, bk_sb, KTt, "k1")
        emit_v(0, "v0")
        emit_v(1, "v1")

        # ---- main pipeline over NCH chunks x NKT ktiles ----
        groups = [(c, i) for c in range(NCH) for i in range(NKT)]
        emit_scores(0, CHUNKS[0][1], 0, 0)
        attnT_ps = den_ps = None
        tail_thunk = None
        avden_prev = None
        for g, (c, i) in enumerate(groups):
            cw0, cww = CHUNKS[c]
            if i == 0:
                attnT_ps = avps.tile([128, PW], f32, name=f"attnT{c}", tag="attnT",
                                     padded_shape=[128, 512])
                den_ps = dnps.tile([128, PW], f32, name=f"den{c}", tag="den",
                                   padded_shape=[128, 512])
            o, sz = KTILES[i]
            P = psb.tile([128, 4 * PW], bf16, name=f"P{c}_{i}", tag="P")
            # The dependency tracker is bounding-box conservative: the
            # strided exp read spans both halves of the score tile, so
            # whichever of {scores(g+1), exp(g)} is emitted LAST waits for
            # the other. Emitting scores(g+1) FIRST puts the short scores op
            # on the exp critical path (instead of exp+AV+den), and the
            # exp(g)-after-scores(g+1) edge is harmless: both are ready.
            if g + 1 < len(groups):
                nc2, ni = groups[g + 1]
                emit_scores(CHUNKS[nc2][0], CHUNKS[nc2][1], ni, (g + 1) % 2)
            emit_exp(cww, sz, g % 2, P, f"e{c}_{i}")
            # tail of chunk c-1 precedes this chunk's first AV (avps/dnps
            # have one buf; the pool WAR edge needs mul already emitted)
            if tail_thunk is not None and i == 1:
                tail_thunk()
                tail_thunk = None
            # deferred by one group so they never gate the next scores:
            # AV/den of group g-1 run on PE while ACT exps group g
            if avden_prev is not None:
                avden_prev()
                avden_prev = None
            avden_prev = (lambda P=P, w=cww, i=i, a=attnT_ps, d=den_ps:
                          (emit_av(P, w, i, a, start=(i == 0), stop=(i == NKT - 1)),
                           emit_den(P, w, i, d, start=(i == 0), stop=(i == NKT - 1))))
            for th in pending.get((c, i), ()):
                th()
            if i == NKT - 1:
                tail_thunk = (lambda c0=cw0, w=cww, a=attnT_ps, d=den_ps, c=c:
                              emit_tail(c0, w, a, d, f"t{c}"))
        avden_prev()
        avden_prev = None
        tail_thunk()

        ctx.close()

    nc.compile()
    return nc


def prepare_in_maps(x1, x2, pos_emb, w_qkv, b_qkv, w_out, b_out):
    import ml_dtypes

    bf16 = ml_dtypes.bfloat16
    x1 = np.asarray(x1, dtype=np.float32)
    x2 = np.asarray(x2, dtype=np.float32)
    pos = np.asarray(pos_emb, dtype=np.float32).reshape(C)
    w_qkv = np.asarray(w_qkv, dtype=np.float32)
    b_qkv = np.asarray(b_qkv, dtype=np.float32)
    w_out = np.asarray(w_out, dtype=np.float32)
    b_out = np.asarray(b_out, dtype=np.float32)

    scale = 1.0 / np.sqrt(np.float32(DH))
    b_eff = b_qkv + w_qkv @ pos
    wq = w_qkv[0:C] * scale
    bq = b_eff[0:C] * scale
    wk = w_qkv[C:2 * C]
    bk = b_eff[C:2 * C]
    wv = w_qkv[2 * C:3 * C]
    bv = b_eff[2 * C:3 * C]

    in_maps = []
    for core in range(8):
        b = core // 2
        g = core % 2
        gsl = slice(GC * g, GC * (g + 1))
        wqkT = np.concatenate([wq[gsl], wk[gsl]], axis=0).T.copy()     # [C, 256]
        wvT = wv[gsl].T.copy()                                         # [C, GC]
        woutT = w_out[:, gsl].T.copy()                                 # [GC, 256]
        bqk = np.stack([bq[gsl], bk[gsl]])[:, :, None].copy()          # [2, 128, 1]
        in_maps.append({
            "x1b": np.ascontiguousarray(x1[b].reshape(C, N)).astype(bf16),
            "x2b": np.ascontiguousarray(x2[b].reshape(C, N)).astype(bf16),
            "wqkT": np.ascontiguousarray(wqkT).astype(bf16),
            "wvT": np.ascontiguousarray(wvT).astype(bf16),
            "woutT": np.ascontiguousarray(woutT).astype(bf16),
            "bqk": np.ascontiguousarray(bqk),
        })
    # out1+out2 folds two tokens, each carrying b_out and the V-bias term
    y_const = 2.0 * (b_out + w_out @ bv)  # [C]
    return in_maps, y_const


def get_nc(repeat=1):
    key = repeat
    if key not in _cache:
        _cache[key] = _build_nc(repeat)
    return _cache[key]


def assemble(per_core_y, y_const):
    out = np.empty((B, C, H, W), dtype=np.float32)
    for b in range(B):
        yb = per_core_y[2 * b] + per_core_y[2 * b + 1]                 # [S, C]
        yf = yb[:N] + yb[N:] + y_const[None, :]                        # [N, C]
        out[b] = yf.T.reshape(C, H, W)
    return out


def kernel(x1, x2, pos_emb, w_qkv, b_qkv, w_out, b_out):
    global LAST_RESULTS, LAST_IN_MAPS
    from concourse.bass_utils import run_bass_kernel_spmd

    in_maps, y_const = prepare_in_maps(x1, x2, pos_emb, w_qkv, b_qkv, w_out, b_out)
    LAST_IN_MAPS = in_maps
    nc = get_nc()
    res = run_bass_kernel_spmd(nc, in_maps, core_ids=list(range(8)))
    LAST_RESULTS = res
    return assemble([res.results[c]["y"] for c in range(8)], y_const)


# revision 32
# speedup vs baseline: 16135.7867x; 1.0197x over previous
"""Trainium2 Bass kernel for EnhancedCrossAttention.

Shapes (hardcoded): B=4, C=256, H=W=28, heads=8, head_dim=32.
Sharding: 8 cores = 4 batches x 2 head-groups (4 heads each core).
Each core computes its batch's QKV (its head-group's Q/K/V), attention for
4 heads, and a partial out-projection (contracting its 128 attention-output
channels). Host sums the two partials per batch, adds the folded bias,
folds the two spatial halves, and reshapes.

Host-side algebraic folds (all exact):
  - pos_emb enters only via the QKV matmul: b_eff = b_qkv + w_qkv @ pos
  - 1/sqrt(dh) folded into Q weights+bias
  - V bias contributes attn_out += b_v (softmax weights sum to 1), folded
    through w_out into a constant added on the host.

v2 design (vs v1): the v1 trace showed PE and ACT strictly alternating at a
~3.9us period per k-tile — the in-order PE queue was [scores_i,
AV_i(waits exp_i), scores_{i+1}], so scores_{i+1} could not run during
exp_i even though its inputs were ready. v2 software-pipelines: the scores
for k-tile group g+1 are emitted into the PE stream BEFORE the AV/den of
group g, so the PE works through them while ACT exps group g.

Hardware constraint (found by bisection): concurrent row-quadrant-packed
matmuls (tile_position=(32h, 0)) must NOT write the same PSUM bank — two
heads' score matmuls writing the same partitions of one bank crash the
exec unit (NRT_EXEC_UNIT_UNRECOVERABLE). So scores use ONE persistent
[128, 2048] tile where head h owns bank h (512 f32 cols), and chunks are
256 q-tokens: group g writes the 256-col half (g%2) of each head's bank.
The half alternation gives double-buffering within 4 banks; attnT (1) +
den (1) + a 2-buf utility pool (2) for QKV/V/out-proj = 8 banks exactly.
All matmul operands are bf16 (1 col/cycle; fp32 is 4) and are cast
host-side so no on-chip round-copy passes are needed. den's stationary
ones operand is [sz, 32], which lands the denominator broadcast across
each head's 32 partitions at identical stream cost, so normalization is
just reciprocal+multiply on DVE (no PE broadcast matmul). Q/K/V
production is deferred into the pipeline through the utility pool so the
exp stream starts ~2us in and QKV hides under early exps.
"""

import numpy as np

B, C, H, W = 4, 256, 28, 28
N = H * W            # 784
S = 2 * N            # 1568 tokens
NH = 8
DH = 32
GH = 4               # heads per group (per core)
GC = GH * DH         # 128 channels per group

PW = 256             # head stride half-unit in the score tile
CHUNKS = [(0, 256), (256, 256), (512, 256), (768, 256), (1024, 256),
          (1280, 256), (1536, 32)]
NCH = len(CHUNKS)    # 7 chunks covering 1568 q-tokens exactly
KTILES = [(i * 128, min(128, S - i * 128)) for i in range((S + 127) // 128)]
NKT = len(KTILES)    # 13 (12 full + one 32-row k-rump)

_cache = {}
LAST_RESULTS = None
LAST_IN_MAPS = None


def _build_nc(repeat=1):
    import concourse.mybir as mybir
    import concourse.tile as tile
    from concourse import bacc
    from contextlib import ExitStack

    f32 = mybir.dt.float32
    bf16 = mybir.dt.bfloat16
    Exp = mybir.ActivationFunctionType.Exp

    nc = bacc.Bacc("TRN2", target_bir_lowering=False, debug=False)

    x1_d = nc.dram_tensor("x1b", [C, N], bf16, kind="ExternalInput")
    x2_d = nc.dram_tensor("x2b", [C, N], bf16, kind="ExternalInput")
    wqk_d = nc.dram_tensor("wqkT", [C, 256], bf16, kind="ExternalInput")
    wv_d = nc.dram_tensor("wvT", [C, GC], bf16, kind="ExternalInput")
    wout_d = nc.dram_tensor("woutT", [GC, 256], bf16, kind="ExternalInput")
    bqk_d = nc.dram_tensor("bqk", [2, 128, 1], f32, kind="ExternalInput")
    y_d = nc.dram_tensor("y", [S, C], f32, kind="ExternalOutput")

    with tile.TileContext(nc) as tc:
      for _rep in range(repeat):
        ctx = ExitStack()
        pp = ctx.enter_context(tc.tile_pool(name="persist", bufs=1))
        sb = ctx.enter_context(tc.tile_pool(name="work", bufs=3))
        psb = ctx.enter_context(tc.tile_pool(name="pwork", bufs=3))
        stps = ctx.enter_context(tc.tile_pool(name="stps", bufs=1, space="PSUM"))
        avps = ctx.enter_context(tc.tile_pool(name="avps", bufs=1, space="PSUM"))
        dnps = ctx.enter_context(tc.tile_pool(name="dnps", bufs=1, space="PSUM"))
        utps = ctx.enter_context(tc.tile_pool(name="utps", bufs=2, space="PSUM"))

        xT = [pp.tile([128, S], bf16, name=f"xT{t}", tag=f"xT{t}") for t in range(2)]
        wqk_sb = [pp.tile([128, 256], bf16, name=f"wqk{t}", tag=f"wqk{t}") for t in range(2)]
        wv_sb = [pp.tile([128, GC], bf16, name=f"wv{t}", tag=f"wv{t}") for t in range(2)]
        wout_sb = pp.tile([128, 256], bf16, name="wout", tag="wout")
        bq_sb = pp.tile([128, 1], f32, name="bq", tag="bq")
        bk_sb = pp.tile([128, 1], f32, name="bk", tag="bk")
        QT = pp.tile([128, S], bf16, name="QT", tag="QT")
        KTt = pp.tile([128, S], bf16, name="KT", tag="KT")
        Vt = [pp.tile([128, GC], bf16, name=f"V{i}", tag=f"V{i}") for i in range(NKT)]
        ones_f = pp.tile([128, 32], f32, name="onesf", tag="onesf")
        ones_b = pp.tile([128, 32], bf16, name="onesb", tag="onesb")
        zc = pp.tile([128, 32], f32, name="zc", tag="zc")

        # the persistent 4-bank score tile: head h owns bank h (512 cols;
        # two 256-col halves, consecutive groups alternate halves — matmul
        # dsts must not cross a PSUM bank boundary)
        st = stps.tile([128, 2048], f32, name="st", tag="st")
        st_h = st[:].rearrange("p (h c) -> p h c", h=4)  # [128, 4, 512]

        # ones via exp(0): exact 1.0 and pre-loads the ACT exp table early
        nc.vector.memset(zc[:], 0.0)
        nc.scalar.activation(ones_f[:], zc[:], Exp)
        nc.vector.tensor_copy(ones_b[:], ones_f[:])

        # --- input DMA (bf16 direct); first 320 x-cols land first so
        # chunk 0's QKV/scores start early ---
        x1v = x1_d[:].rearrange("(t p) f -> t p f", p=128)
        x2v = x2_d[:].rearrange("(t p) f -> t p f", p=128)
        wqkv = wqk_d[:].rearrange("(t p) f -> t p f", p=128)
        wvv = wv_d[:].rearrange("(t p) f -> t p f", p=128)
        # critical path first: weights + the x columns chunk 0 needs
        # (Q c0 / K c0 contract x1[:, 0:256] on both C-halves), so the
        # first scores/exp start ~3us in instead of queueing behind the
        # ~1MB bulk x transfer.
        for t in range(2):
            nc.sync.dma_start(wqk_sb[t][:], wqkv[t])
        nc.sync.dma_start(bq_sb[:], bqk_d[0])
        nc.sync.dma_start(bk_sb[:], bqk_d[1])
        for t in range(2):
            nc.sync.dma_start(xT[t][:, 0:PW], x1v[t][:, 0:PW])
        for t in range(2):
            nc.sync.dma_start(wv_sb[t][:], wvv[t])
            nc.sync.dma_start(xT[t][:, PW:2 * PW], x1v[t][:, PW:2 * PW])
        for t in range(2):
            nc.sync.dma_start(xT[t][:, 2 * PW:N], x1v[t][:, 2 * PW:N])
            nc.sync.dma_start(xT[t][:, N:N + PW], x2v[t][:, 0:PW])
            nc.sync.dma_start(xT[t][:, N + PW:S], x2v[t][:, PW:N])
        nc.sync.dma_start(wout_sb[:], wout_d[:])

        # ---- emit helpers ----
        def ut_tile(nm):
            # [128, 512] f32 = 1 bank; pool has 2 bufs
            return utps.tile([128, 512], f32, name=nm, tag="ut")

        def emit_qk(c0, c1, m, bias_t, out_t, nm):
            w = c1 - c0
            ps = ut_tile(nm)
            for t in range(2):
                nc.tensor.matmul(
                    ps[:, :w], wqk_sb[t][:, 128 * m:128 * m + 128], xT[t][:, c0:c1],
                    start=(t == 0), stop=(t == 1),
                )
            nc.vector.tensor_scalar_add(out_t[:, c0:c1], ps[:, :w], bias_t[:])

        def emit_v(i, nm):
            o, sz = KTILES[i]
            ps = ut_tile(nm)
            for t in range(2):
                nc.tensor.matmul(
                    ps[:sz, 0:GC], xT[t][:, o:o + sz], wv_sb[t][:],
                    start=(t == 0), stop=(t == 1),
                )
            nc.vector.tensor_copy(Vt[i][:sz, :], ps[:sz, 0:GC])

        def emit_scores(c0, w, i, half, off=0):
            # head h -> its own PSUM bank (concurrent row-packed matmuls
            # must not share a bank); halves alternate for double-buffering
            o, sz = KTILES[i]
            for h in range(4):
                base = 512 * h + PW * half + off
                nc.tensor.matmul(
                    st[:sz, base:base + w],
                    KTt[32 * h:32 * h + 32, o:o + sz],
                    QT[32 * h:32 * h + 32, c0:c0 + w],
                    start=True, stop=True,
                    tile_position=(32 * h, 0),
                )

        def emit_exp(w, sz, half, P, nm, off=0):
            lo = PW * half + off
            stv = st_h[:sz, :, lo:lo + w]
            pv = P[:sz, :].rearrange("p (h c) -> p h c", h=4)[:, :, 0:w]
            nc.scalar.activation(pv, stv, Exp)

        def emit_av(P, w, i, attnT_ps, start, stop):
            o, sz = KTILES[i]
            for h in range(4):
                nc.tensor.matmul(
                    attnT_ps[32 * h:32 * h + 32, :w],
                    Vt[i][:sz, 32 * h:32 * h + 32],
                    P[:sz, PW * h:PW * h + w],
                    start=start, stop=stop, skip_group_check=True,
                    tile_position=(0, 32 * h),
                )

        def emit_den(P, w, i, den_ps, start, stop):
            o, sz = KTILES[i]
            for h in range(4):
                nc.tensor.matmul(
                    den_ps[32 * h:32 * h + 32, 0:w],
                    ones_b[:sz, :],
                    P[:sz, PW * h:PW * h + w],
                    start=start, stop=stop, skip_group_check=True,
                    tile_position=(0, 32 * h),
                )

        def emit_tail(c0, w, attnT_ps, den_ps, nm):
            # den is broadcast across each head's 32 partitions (ones lhsT
            # is [sz, 32]), so normalization is reciprocal + multiply only.
            recip_f = sb.tile([128, PW], f32, name=f"rf{nm}", tag="recipf")
            nc.vector.reciprocal_approx_fast(recip_f[:, :w], den_ps[:, :w])
            attn_sb = sb.tile([128, PW], bf16, name=f"at{nm}", tag="attnsb")
            nc.vector.tensor_mul(attn_sb[:, :w], attnT_ps[:, :w], recip_f[:, :w])
            nsub = (w + 127) // 128
            for s4 in range(nsub):
                ssz = min(128, w - 128 * s4)
                off = 128 * s4
                yp = ut_tile(f"yp{nm}{s4}")
                nc.tensor.matmul(
                    yp[:ssz, 0:256], attn_sb[:, off:off + ssz], wout_sb[:],
                    start=True, stop=True,
                )
                ysb = sb.tile([128, 256], f32, name=f"ys{nm}{s4}", tag="ysb")
                nc.vector.tensor_copy(ysb[:ssz, :], yp[:ssz, 0:256])
                nc.sync.dma_start(y_d[c0 + off:c0 + off + ssz, :], ysb[:ssz, :])

        # ---- deferred production schedule ----
        # K chunk j covers score k-tiles 2j, 2j+1 -> keep 3-4 tiles ahead.
        # V k-tile j is consumed by group (0, j) -> produce at (0, j-2).
        # Q chunk c+1 is consumed from chunk c+1 -> produce mid-chunk c.
        pending = {}

        def defer(c, i, th):
            pending.setdefault((c, i), []).append(th)

        # K chunk j covers tokens [c0, c0+w); first score k-tile touching
        # it is floor(c0/128) — emit 2 k-tiles ahead.
        for j in range(1, NCH):
            c0j, wj = CHUNKS[j]
            ii = max(0, c0j // 128 - 2)
            if j == 1:
                continue  # K1 in the prologue
            defer(0, ii, lambda c0j=c0j, wj=wj, j=j: emit_qk(
                c0j, c0j + wj, 1, bk_sb, KTt, f"k{j}"))
        for j in range(2, NKT):
            defer(0, j - 2, lambda j=j: emit_v(j, f"v{j}"))
        for c in range(NCH - 1):
            defer(c, 6, lambda c=c: emit_qk(
                CHUNKS[c + 1][0], CHUNKS[c + 1][0] + CHUNKS[c + 1][1],
                0, bq_sb, QT, f"q{c+1}"))

        # ---- prologue ----
        emit_qk(0, PW, 0, bq_sb, QT, "q0")
        emit_qk(0, PW, 1, bk_sb, KTt, "k0")
        emit_qk(PW, 2 * PW, 1, bk_sb, KTt, "k1")
        emit_v(0, "v0")
        emit_v(1, "v1")

        # ---- main pipeline over NCH chunks x NKT ktiles ----
        groups = [(c, i) for c in range(NCH) for i in range(NKT)]
        emit_scores(0, CHUNKS[0][1], 0, 0)
        attnT_ps = den_ps = None
        tail_thunk = None
        avden_prev = None
        for g, (c, i) in enumerate(groups):
            cw0, cww = CHUNKS[c]
            if i == 0:
                attnT_ps = avps.tile([128, PW], f32, name=f"attnT{c}", tag="attnT",
                                     padded_shape=[128, 512])
                den_ps = dnps.tile([128, PW], f32, name=f"den{c}", tag="den",
                                   padded_shape=[128, 512])
            o, sz = KTILES[i]
            P = psb.tile([128, 4 * PW], bf16, name=f"P{c}_{i}", tag="P")
            # The dependency tracker is bounding-box conservative: the
            # strided exp read spans both halves of the score tile, so
            # whichever of {scores(g+1), exp(g)} is emitted LAST waits for
            # the other. Emitting scores(g+1) FIRST puts the short scores op
            # on the exp critical path (instead of exp+AV+den), and the
            # exp(g)-after-scores(g+1) edge is harmless: both are ready.
            if g + 1 < len(groups):
                nc2, ni = groups[g + 1]
                emit_scores(CHUNKS[nc2][0], CHUNKS[nc2][1], ni, (g + 1) % 2)
            emit_exp(cww, sz, g % 2, P, f"e{c}_{i}")
            # tail of chunk c-1 precedes this chunk's first AV (avps/dnps
            # have one buf; the pool WAR edge needs mul already emitted)
            if tail_thunk is not None and i == 1:
                tail_thunk()
                tail_thunk = None
            # deferred by one group so they never gate the next scores:
            # AV/den of group g-1 run on PE while ACT exps group g
            if avden_prev is not None:
                avden_prev()
                avden_prev = None
            avden_prev = (lambda P=P, w=cww, i=i, a=attnT_ps, d=den_ps:
                          (emit_av(P, w, i, a, start=(i == 0), stop=(i == NKT - 1)),
                           emit_den(P, w, i, d, start=(i == 0), stop=(i == NKT - 1))))
            for th in pending.get((c, i), ()):
                th()
            if i == NKT - 1:
                tail_thunk = (lambda c0=cw0, w=cww, a=attnT_ps, d=den_ps, c=c:
                              emit_tail(c0, w, a, d, f"t{c}"))
        avden_prev()
        avden_prev = None
        tail_thunk()

        ctx.close()

    nc.compile()
    return nc


def prepare_in_maps(x1, x2, pos_emb, w_qkv, b_qkv, w_out, b_out):
    import ml_dtypes

    bf16 = ml_dtypes.bfloat16
    x1 = np.asarray(x1, dtype=np.float32)
    x2 = np.asarray(x2, dtype=np.float32)
    pos = np.asarray(pos_emb, dtype=np.float32).reshape(C)
    w_qkv = np.asarray(w_qkv, dtype=np.float32)
    b_qkv = np.asarray(b_qkv, dtype=np.float32)
    w_out = np.asarray(w_out, dtype=np.float32)
    b_out = np.asarray(b_out, dtype=np.float32)

    scale = 1.0 / np.sqrt(np.float32(DH))
    b_eff = b_qkv + w_qkv @ pos
    wq = w_qkv[0:C] * scale
    bq = b_eff[0:C] * scale
    wk = w_qkv[C:2 * C]
    bk = b_eff[C:2 * C]
    wv = w_qkv[2 * C:3 * C]
    bv = b_eff[2 * C:3 * C]

    in_maps = []
    for core in range(8):
        b = core // 2
        g = core % 2
        gsl = slice(GC * g, GC * (g + 1))
        wqkT = np.concatenate([wq[gsl], wk[gsl]], axis=0).T.copy()     # [C, 256]
        wvT = wv[gsl].T.copy()                                         # [C, GC]
        woutT = w_out[:, gsl].T.copy()                                 # [GC, 256]
        bqk = np.stack([bq[gsl], bk[gsl]])[:, :, None].copy()          # [2, 128, 1]
        in_maps.append({
            "x1b": np.ascontiguousarray(x1[b].reshape(C, N)).astype(bf16),
            "x2b": np.ascontiguousarray(x2[b].reshape(C, N)).astype(bf16),
            "wqkT": np.ascontiguousarray(wqkT).astype(bf16),
            "wvT": np.ascontiguousarray(wvT).astype(bf16),
            "woutT": np.ascontiguousarray(woutT).astype(bf16),
            "bqk": np.ascontiguousarray(bqk),
        })
    # out1+out2 folds two tokens, each carrying b_out and the V-bias term
    y_const = 2.0 * (b_out + w_out @ bv)  # [C]
    return in_maps, y_const


def get_nc(repeat=1):
    key = repeat
    if key not in _cache:
        _cache[key] = _build_nc(repeat)
    return _cache[key]


def assemble(per_core_y, y_const):
    out = np.empty((B, C, H, W), dtype=np.float32)
    for b in range(B):
        yb = per_core_y[2 * b] + per_core_y[2 * b + 1]                 # [S, C]
        yf = yb[:N] + yb[N:] + y_const[None, :]                        # [N, C]
        out[b] = yf.T.reshape(C, H, W)
    return out


def kernel(x1, x2, pos_emb, w_qkv, b_qkv, w_out, b_out):
    global LAST_RESULTS, LAST_IN_MAPS
    from concourse.bass_utils import run_bass_kernel_spmd

    in_maps, y_const = prepare_in_maps(x1, x2, pos_emb, w_qkv, b_qkv, w_out, b_out)
    LAST_IN_MAPS = in_maps
    nc = get_nc()
    res = run_bass_kernel_spmd(nc, in_maps, core_ids=list(range(8)))
    LAST_RESULTS = res
    return assemble([res.results[c]["y"] for c in range(8)], y_const)


# revision 36
# speedup vs baseline: 18924.5931x; 1.1728x over previous
"""Trainium2 Bass kernel for EnhancedCrossAttention.

Shapes (hardcoded): B=4, C=256, H=W=28, heads=8, head_dim=32.
Sharding: 8 cores = 4 batches x 2 head-groups (4 heads each core).
Each core computes its batch's QKV (its head-group's Q/K/V), attention for
4 heads, and a partial out-projection (contracting its 128 attention-output
channels). Host sums the two partials per batch, adds the folded bias,
folds the two spatial halves, and reshapes.

Host-side algebraic folds (all exact):
  - pos_emb enters only via the QKV matmul: b_eff = b_qkv + w_qkv @ pos
  - 1/sqrt(dh) folded into Q weights+bias
  - V bias contributes attn_out += b_v (softmax weights sum to 1), folded
    through w_out into a constant added on the host.

v2 design (vs v1): the v1 trace showed PE and ACT strictly alternating at a
~3.9us period per k-tile — the in-order PE queue was [scores_i,
AV_i(waits exp_i), scores_{i+1}], so scores_{i+1} could not run during
exp_i even though its inputs were ready. v2 software-pipelines: the scores
for k-tile group g+1 are emitted into the PE stream BEFORE the AV/den of
group g, so the PE works through them while ACT exps group g.

Hardware constraint (found by bisection): concurrent row-quadrant-packed
matmuls (tile_position=(32h, 0)) must NOT write the same PSUM bank — two
heads' score matmuls writing the same partitions of one bank crash the
exec unit (NRT_EXEC_UNIT_UNRECOVERABLE). So scores use ONE persistent
[128, 2048] tile where head h owns bank h (512 f32 cols), and chunks are
256 q-tokens: group g writes the 256-col half (g%2) of each head's bank.
The half alternation gives double-buffering within 4 banks; attnT (1) +
den (1) + a 2-buf utility pool (2) for QKV/V/out-proj = 8 banks exactly.
All matmul operands are bf16 (1 col/cycle; fp32 is 4) and are cast
host-side so no on-chip round-copy passes are needed. den's stationary
ones operand is [sz, 32], which lands the denominator broadcast across
each head's 32 partitions at identical stream cost, so normalization is
just reciprocal+multiply on DVE (no PE broadcast matmul). Q/K/V
production is deferred into the pipeline through the utility pool so the
exp stream starts ~2us in and QKV hides under early exps.
"""

import numpy as np

B, C, H, W = 4, 256, 28, 28
N = H * W            # 784
S = 2 * N            # 1568 tokens
NH = 8
DH = 32
GH = 4               # heads per group (per core)
GC = GH * DH         # 128 channels per group

PW = 512             # head slot width in the score tile (full bank)
CHUNKS = [(0, 512), (512, 512), (1024, 512), (1536, 32)]
NCH = len(CHUNKS)    # 4 chunks covering 1568 q-tokens exactly
KTILES = [(i * 128, min(128, S - i * 128)) for i in range((S + 127) // 128)]
NKT = len(KTILES)    # 13 (12 full + one 32-row k-rump)

_cache = {}
LAST_RESULTS = None
LAST_IN_MAPS = None


def _build_nc(repeat=1):
    import concourse.mybir as mybir
    import concourse.tile as tile
    from concourse import bacc
    from contextlib import ExitStack

    f32 = mybir.dt.float32
    bf16 = mybir.dt.bfloat16
    Exp = mybir.ActivationFunctionType.Exp

    nc = bacc.Bacc("TRN2", target_bir_lowering=False, debug=False)

    x1_d = nc.dram_tensor("x1b", [C, N], bf16, kind="ExternalInput")
    x2_d = nc.dram_tensor("x2b", [C, N], bf16, kind="ExternalInput")
    wqk_d = nc.dram_tensor("wqkT", [C, 256], bf16, kind="ExternalInput")
    wv_d = nc.dram_tensor("wvT", [C, GC], bf16, kind="ExternalInput")
    wout_d = nc.dram_tensor("woutT", [GC, 256], bf16, kind="ExternalInput")
    bqk_d = nc.dram_tensor("bqk", [2, 128, 1], f32, kind="ExternalInput")
    y_d = nc.dram_tensor("y", [S, C], f32, kind="ExternalOutput")

    with tile.TileContext(nc) as tc:
      for _rep in range(repeat):
        ctx = ExitStack()
        pp = ctx.enter_context(tc.tile_pool(name="persist", bufs=1))
        sb = ctx.enter_context(tc.tile_pool(name="work", bufs=3))
        psb = ctx.enter_context(tc.tile_pool(name="pwork", bufs=3))
        stps = ctx.enter_context(tc.tile_pool(name="stps", bufs=1, space="PSUM"))
        avps = ctx.enter_context(tc.tile_pool(name="avps", bufs=1, space="PSUM"))
        dnps = ctx.enter_context(tc.tile_pool(name="dnps", bufs=1, space="PSUM"))
        utps = ctx.enter_context(tc.tile_pool(name="utps", bufs=2, space="PSUM"))

        xT = [pp.tile([128, S], bf16, name=f"xT{t}", tag=f"xT{t}") for t in range(2)]
        wqk_sb = [pp.tile([128, 256], bf16, name=f"wqk{t}", tag=f"wqk{t}") for t in range(2)]
        wv_sb = [pp.tile([128, GC], bf16, name=f"wv{t}", tag=f"wv{t}") for t in range(2)]
        wout_sb = pp.tile([128, 256], bf16, name="wout", tag="wout")
        bq_sb = pp.tile([128, 1], f32, name="bq", tag="bq")
        bk_sb = pp.tile([128, 1], f32, name="bk", tag="bk")
        QT = pp.tile([128, S], bf16, name="QT", tag="QT")
        KTt = pp.tile([128, S], bf16, name="KT", tag="KT")
        Vt = [pp.tile([128, GC], bf16, name=f"V{i}", tag=f"V{i}") for i in range(NKT)]
        ones_f = pp.tile([128, 32], f32, name="onesf", tag="onesf")
        ones_b = pp.tile([128, 32], bf16, name="onesb", tag="onesb")
        zc = pp.tile([128, 32], f32, name="zc", tag="zc")

        # the persistent 4-bank score tile: head h owns bank h (512 cols;
        # two 256-col halves, consecutive groups alternate halves — matmul
        # dsts must not cross a PSUM bank boundary)
        st = stps.tile([128, 2048], f32, name="st", tag="st")
        st_h = st[:].rearrange("p (h c) -> p h c", h=4)  # [128, 4, 512]

        # ones via exp(0): exact 1.0 and pre-loads the ACT exp table early
        nc.vector.memset(zc[:], 0.0)
        nc.scalar.activation(ones_f[:], zc[:], Exp)
        nc.vector.tensor_copy(ones_b[:], ones_f[:])

        # --- input DMA (bf16 direct); first 320 x-cols land first so
        # chunk 0's QKV/scores start early ---
        x1v = x1_d[:].rearrange("(t p) f -> t p f", p=128)
        x2v = x2_d[:].rearrange("(t p) f -> t p f", p=128)
        wqkv = wqk_d[:].rearrange("(t p) f -> t p f", p=128)
        wvv = wv_d[:].rearrange("(t p) f -> t p f", p=128)
        # critical path first: weights + the x columns chunk 0 needs
        # (Q c0 / K c0 contract x1[:, 0:256] on both C-halves), so the
        # first scores/exp start ~3us in instead of queueing behind the
        # ~1MB bulk x transfer.
        for t in range(2):
            nc.sync.dma_start(wqk_sb[t][:], wqkv[t])
        nc.sync.dma_start(bq_sb[:], bqk_d[0])
        nc.sync.dma_start(bk_sb[:], bqk_d[1])
        for t in range(2):
            nc.sync.dma_start(xT[t][:, 0:512], x1v[t][:, 0:512])
        for t in range(2):
            nc.sync.dma_start(wv_sb[t][:], wvv[t])
            nc.sync.dma_start(xT[t][:, 512:N], x1v[t][:, 512:N])
        for t in range(2):
            nc.sync.dma_start(xT[t][:, N:N + 512], x2v[t][:, 0:512])
            nc.sync.dma_start(xT[t][:, N + 512:S], x2v[t][:, 512:N])
        nc.sync.dma_start(wout_sb[:], wout_d[:])

        # ---- emit helpers ----
        def ut_tile(nm):
            # [128, 512] f32 = 1 bank; pool has 2 bufs
            return utps.tile([128, 512], f32, name=nm, tag="ut")

        def emit_qk(c0, c1, m, bias_t, out_t, nm):
            w = c1 - c0
            ps = ut_tile(nm)
            for t in range(2):
                nc.tensor.matmul(
                    ps[:, :w], wqk_sb[t][:, 128 * m:128 * m + 128], xT[t][:, c0:c1],
                    start=(t == 0), stop=(t == 1),
                )
            nc.vector.tensor_scalar_add(out_t[:, c0:c1], ps[:, :w], bias_t[:])

        def emit_v(i, nm):
            o, sz = KTILES[i]
            ps = ut_tile(nm)
            for t in range(2):
                nc.tensor.matmul(
                    ps[:sz, 0:GC], xT[t][:, o:o + sz], wv_sb[t][:],
                    start=(t == 0), stop=(t == 1),
                )
            nc.vector.tensor_copy(Vt[i][:sz, :], ps[:sz, 0:GC])

        def emit_scores(c0, w, i, half, off=0):
            # head h -> its own PSUM bank (concurrent row-packed matmuls
            # must not share a bank). No half alternation: the conservative
            # bbox dependency serializes scores<->exp anyway, so each head
            # uses its full 512-col bank and groups are 2x wider (fewer
            # chain links, fewer per-instr overheads).
            o, sz = KTILES[i]
            for h in range(4):
                base = 512 * h + off
                nc.tensor.matmul(
                    st[:sz, base:base + w],
                    KTt[32 * h:32 * h + 32, o:o + sz],
                    QT[32 * h:32 * h + 32, c0:c0 + w],
                    start=True, stop=True,
                    tile_position=(32 * h, 0),
                )

        def emit_exp(w, sz, half, P, nm, off=0):
            stv = st_h[:sz, :, off:off + w]
            if w == PW:
                nc.scalar.activation(P[:sz, :], st[:sz, :], Exp)
            else:
                pv = P[:sz, :].rearrange("p (h c) -> p h c", h=4)[:, :, 0:w]
                nc.scalar.activation(pv, stv, Exp)

        def emit_av(P, w, i, attnT_ps, start, stop):
            o, sz = KTILES[i]
            for h in range(4):
                nc.tensor.matmul(
                    attnT_ps[32 * h:32 * h + 32, :w],
                    Vt[i][:sz, 32 * h:32 * h + 32],
                    P[:sz, PW * h:PW * h + w],
                    start=start, stop=stop, skip_group_check=True,
                    tile_position=(0, 32 * h),
                )

        def emit_den(P, w, i, den_ps, start, stop):
            o, sz = KTILES[i]
            for h in range(4):
                nc.tensor.matmul(
                    den_ps[32 * h:32 * h + 32, 0:w],
                    ones_b[:sz, :],
                    P[:sz, PW * h:PW * h + w],
                    start=start, stop=stop, skip_group_check=True,
                    tile_position=(0, 32 * h),
                )

        def emit_tail(c0, w, attnT_ps, den_ps, nm):
            # den is broadcast across each head's 32 partitions (ones lhsT
            # is [sz, 32]), so normalization is reciprocal + multiply only.
            recip_f = sb.tile([128, PW], f32, name=f"rf{nm}", tag="recipf")
            nc.vector.reciprocal_approx_fast(recip_f[:, :w], den_ps[:, :w])
            attn_sb = sb.tile([128, PW], bf16, name=f"at{nm}", tag="attnsb")
            nc.vector.tensor_mul(attn_sb[:, :w], attnT_ps[:, :w], recip_f[:, :w])
            nsub = (w + 127) // 128
            for s4 in range(nsub):
                ssz = min(128, w - 128 * s4)
                off = 128 * s4
                yp = ut_tile(f"yp{nm}{s4}")
                nc.tensor.matmul(
                    yp[:ssz, 0:256], attn_sb[:, off:off + ssz], wout_sb[:],
                    start=True, stop=True,
                )
                ysb = sb.tile([128, 256], f32, name=f"ys{nm}{s4}", tag="ysb")
                nc.vector.tensor_copy(ysb[:ssz, :], yp[:ssz, 0:256])
                nc.sync.dma_start(y_d[c0 + off:c0 + off + ssz, :], ysb[:ssz, :])

        # ---- deferred production schedule ----
        # K chunk j covers score k-tiles 2j, 2j+1 -> keep 3-4 tiles ahead.
        # V k-tile j is consumed by group (0, j) -> produce at (0, j-2).
        # Q chunk c+1 is consumed from chunk c+1 -> produce mid-chunk c.
        pending = {}

        def defer(c, i, th):
            pending.setdefault((c, i), []).append(th)

        # K chunk j covers tokens [c0, c0+w); first score k-tile touching
        # it is floor(c0/128) — emit 2-3 k-tiles ahead.
        for j in range(1, NCH):
            c0j, wj = CHUNKS[j]
            ii = max(0, c0j // 128 - 3)
            defer(0, ii, lambda c0j=c0j, wj=wj, j=j: emit_qk(
                c0j, c0j + wj, 1, bk_sb, KTt, f"k{j}"))
        for j in range(2, NKT):
            defer(0, j - 2, lambda j=j: emit_v(j, f"v{j}"))
        for c in range(NCH - 1):
            defer(c, 6, lambda c=c: emit_qk(
                CHUNKS[c + 1][0], CHUNKS[c + 1][0] + CHUNKS[c + 1][1],
                0, bq_sb, QT, f"q{c+1}"))

        # ---- prologue ----
        emit_qk(0, 512, 0, bq_sb, QT, "q0")
        emit_qk(0, 512, 1, bk_sb, KTt, "k0")
        emit_v(0, "v0")
        emit_v(1, "v1")

        # ---- main pipeline over NCH chunks x NKT ktiles ----
        groups = [(c, i) for c in range(NCH) for i in range(NKT)]
        emit_scores(0, CHUNKS[0][1], 0, 0)
        attnT_ps = den_ps = None
        tail_thunk = None
        avden_prev = None
        for g, (c, i) in enumerate(groups):
            cw0, cww = CHUNKS[c]
            if i == 0:
                attnT_ps = avps.tile([128, PW], f32, name=f"attnT{c}", tag="attnT",
                                     padded_shape=[128, 512])
                den_ps = dnps.tile([128, PW], f32, name=f"den{c}", tag="den",
                                   padded_shape=[128, 512])
            o, sz = KTILES[i]
            P = psb.tile([128, 4 * PW], bf16, name=f"P{c}_{i}", tag="P")
            # Without half alternation scores(g+1) overwrite the slot
            # exp(g) reads, so exp MUST be emitted first (the tracker pairs
            # each read with the last prior write). The resulting chain
            # exp(g) -> scores(g+1) -> exp(g+1) is the same serialization
            # the bbox edges forced anyway; wider groups amortize it.
            emit_exp(cww, sz, g % 2, P, f"e{c}_{i}")
            if g + 1 < len(groups):
                nc2, ni = groups[g + 1]
                emit_scores(CHUNKS[nc2][0], CHUNKS[nc2][1], ni, (g + 1) % 2)
            # tail of chunk c-1 precedes this chunk's first AV (avps/dnps
            # have one buf; the pool WAR edge needs mul already emitted)
            if tail_thunk is not None and i == 1:
                tail_thunk()
                tail_thunk = None
            # deferred by one group so they never gate the next scores:
            # AV/den of group g-1 run on PE while ACT exps group g
            if avden_prev is not None:
                avden_prev()
                avden_prev = None
            avden_prev = (lambda P=P, w=cww, i=i, a=attnT_ps, d=den_ps:
                          (emit_av(P, w, i, a, start=(i == 0), stop=(i == NKT - 1)),
                           emit_den(P, w, i, d, start=(i == 0), stop=(i == NKT - 1))))
            for th in pending.get((c, i), ()):
                th()
            if i == NKT - 1:
                tail_thunk = (lambda c0=cw0, w=cww, a=attnT_ps, d=den_ps, c=c:
                              emit_tail(c0, w, a, d, f"t{c}"))
        avden_prev()
        avden_prev = None
        tail_thunk()

        ctx.close()

    nc.compile()
    return nc


def prepare_in_maps(x1, x2, pos_emb, w_qkv, b_qkv, w_out, b_out):
    import ml_dtypes

    bf16 = ml_dtypes.bfloat16
    x1 = np.asarray(x1, dtype=np.float32)
    x2 = np.asarray(x2, dtype=np.float32)
    pos = np.asarray(pos_emb, dtype=np.float32).reshape(C)
    w_qkv = np.asarray(w_qkv, dtype=np.float32)
    b_qkv = np.asarray(b_qkv, dtype=np.float32)
    w_out = np.asarray(w_out, dtype=np.float32)
    b_out = np.asarray(b_out, dtype=np.float32)

    scale = 1.0 / np.sqrt(np.float32(DH))
    b_eff = b_qkv + w_qkv @ pos
    wq = w_qkv[0:C] * scale
    bq = b_eff[0:C] * scale
    wk = w_qkv[C:2 * C]
    bk = b_eff[C:2 * C]
    wv = w_qkv[2 * C:3 * C]
    bv = b_eff[2 * C:3 * C]

    in_maps = []
    for core in range(8):
        b = core // 2
        g = core % 2
        gsl = slice(GC * g, GC * (g + 1))
        wqkT = np.concatenate([wq[gsl], wk[gsl]], axis=0).T.copy()     # [C, 256]
        wvT = wv[gsl].T.copy()                                         # [C, GC]
        woutT = w_out[:, gsl].T.copy()                                 # [GC, 256]
        bqk = np.stack([bq[gsl], bk[gsl]])[:, :, None].copy()          # [2, 128, 1]
        in_maps.append({
            "x1b": np.ascontiguousarray(x1[b].reshape(C, N)).astype(bf16),
            "x2b": np.ascontiguousarray(x2[b].reshape(C, N)).astype(bf16),
            "wqkT": np.ascontiguousarray(wqkT).astype(bf16),
            "wvT": np.ascontiguousarray(wvT).astype(bf16),
            "woutT": np.ascontiguousarray(woutT).astype(bf16),
            "bqk": np.ascontiguousarray(bqk),
        })
    # out1+out2 folds two tokens, each carrying b_out and the V-bias term
    y_const = 2.0 * (b_out + w_out @ bv)  # [C]
    return in_maps, y_const


def get_nc(repeat=1):
    key = repeat
    if key not in _cache:
        _cache[key] = _build_nc(repeat)
    return _cache[key]


def assemble(per_core_y, y_const):
    out = np.empty((B, C, H, W), dtype=np.float32)
    for b in range(B):
        yb = per_core_y[2 * b] + per_core_y[2 * b + 1]                 # [S, C]
        yf = yb[:N] + yb[N:] + y_const[None, :]                        # [N, C]
        out[b] = yf.T.reshape(C, H, W)
    return out


def kernel(x1, x2, pos_emb, w_qkv, b_qkv, w_out, b_out):
    global LAST_RESULTS, LAST_IN_MAPS
    from concourse.bass_utils import run_bass_kernel_spmd

    in_maps, y_const = prepare_in_maps(x1, x2, pos_emb, w_qkv, b_qkv, w_out, b_out)
    LAST_IN_MAPS = in_maps
    nc = get_nc()
    res = run_bass_kernel_spmd(nc, in_maps, core_ids=list(range(8)))
    LAST_RESULTS = res
    return assemble([res.results[c]["y"] for c in range(8)], y_const)


# revision 37
# speedup vs baseline: 19281.9249x; 1.0189x over previous
"""Trainium2 Bass kernel for EnhancedCrossAttention.

Shapes (hardcoded): B=4, C=256, H=W=28, heads=8, head_dim=32.
Sharding: 8 cores = 4 batches x 2 head-groups (4 heads each core).
Each core computes its batch's QKV (its head-group's Q/K/V), attention for
4 heads, and a partial out-projection (contracting its 128 attention-output
channels). Host sums the two partials per batch, adds the folded bias,
folds the two spatial halves, and reshapes.

Host-side algebraic folds (all exact):
  - pos_emb enters only via the QKV matmul: b_eff = b_qkv + w_qkv @ pos
  - 1/sqrt(dh) folded into Q weights+bias
  - V bias contributes attn_out += b_v (softmax weights sum to 1), folded
    through w_out into a constant added on the host.

v2 design (vs v1): the v1 trace showed PE and ACT strictly alternating at a
~3.9us period per k-tile — the in-order PE queue was [scores_i,
AV_i(waits exp_i), scores_{i+1}], so scores_{i+1} could not run during
exp_i even though its inputs were ready. v2 software-pipelines: the scores
for k-tile group g+1 are emitted into the PE stream BEFORE the AV/den of
group g, so the PE works through them while ACT exps group g.

Hardware constraint (found by bisection): concurrent row-quadrant-packed
matmuls (tile_position=(32h, 0)) must NOT write the same PSUM bank — two
heads' score matmuls writing the same partitions of one bank crash the
exec unit (NRT_EXEC_UNIT_UNRECOVERABLE). So scores use ONE persistent
[128, 2048] tile where head h owns bank h (512 f32 cols), and chunks are
256 q-tokens: group g writes the 256-col half (g%2) of each head's bank.
The half alternation gives double-buffering within 4 banks; attnT (1) +
den (1) + a 2-buf utility pool (2) for QKV/V/out-proj = 8 banks exactly.
All matmul operands are bf16 (1 col/cycle; fp32 is 4) and are cast
host-side so no on-chip round-copy passes are needed. den's stationary
ones operand is [sz, 32], which lands the denominator broadcast across
each head's 32 partitions at identical stream cost, so normalization is
just reciprocal+multiply on DVE (no PE broadcast matmul). Q/K/V
production is deferred into the pipeline through the utility pool so the
exp stream starts ~2us in and QKV hides under early exps.
"""

import numpy as np

B, C, H, W = 4, 256, 28, 28
N = H * W            # 784
S = 2 * N            # 1568 tokens
NH = 8
DH = 32
GH = 4               # heads per group (per core)
GC = GH * DH         # 128 channels per group

PW = 512             # head slot width in the score tile (full bank)
CHUNKS = [(0, 512), (512, 512), (1024, 512), (1536, 32)]
NCH = len(CHUNKS)    # 4 chunks covering 1568 q-tokens exactly
KTILES = [(i * 128, min(128, S - i * 128)) for i in range((S + 127) // 128)]
NKT = len(KTILES)    # 13 (12 full + one 32-row k-rump)

_cache = {}
LAST_RESULTS = None
LAST_IN_MAPS = None


def _build_nc(repeat=1):
    import concourse.mybir as mybir
    import concourse.tile as tile
    from concourse import bacc
    from contextlib import ExitStack

    f32 = mybir.dt.float32
    bf16 = mybir.dt.bfloat16
    Exp = mybir.ActivationFunctionType.Exp

    nc = bacc.Bacc("TRN2", target_bir_lowering=False, debug=False)

    x1_d = nc.dram_tensor("x1b", [C, N], bf16, kind="ExternalInput")
    x2_d = nc.dram_tensor("x2b", [C, N], bf16, kind="ExternalInput")
    wqk_d = nc.dram_tensor("wqkT", [C, 256], bf16, kind="ExternalInput")
    wv_d = nc.dram_tensor("wvT", [C, GC], bf16, kind="ExternalInput")
    wout_d = nc.dram_tensor("woutT", [GC, 256], bf16, kind="ExternalInput")
    bqk_d = nc.dram_tensor("bqk", [2, 128, 1], f32, kind="ExternalInput")
    y_d = nc.dram_tensor("y", [S, C], f32, kind="ExternalOutput")

    with tile.TileContext(nc) as tc:
      for _rep in range(repeat):
        ctx = ExitStack()
        pp = ctx.enter_context(tc.tile_pool(name="persist", bufs=1))
        sb = ctx.enter_context(tc.tile_pool(name="work", bufs=3))
        psb = ctx.enter_context(tc.tile_pool(name="pwork", bufs=3))
        stps = ctx.enter_context(tc.tile_pool(name="stps", bufs=1, space="PSUM"))
        avps = ctx.enter_context(tc.tile_pool(name="avps", bufs=1, space="PSUM"))
        dnps = ctx.enter_context(tc.tile_pool(name="dnps", bufs=1, space="PSUM"))
        utps = ctx.enter_context(tc.tile_pool(name="utps", bufs=2, space="PSUM"))

        xT = [pp.tile([128, S], bf16, name=f"xT{t}", tag=f"xT{t}") for t in range(2)]
        wqk_sb = [pp.tile([128, 256], bf16, name=f"wqk{t}", tag=f"wqk{t}") for t in range(2)]
        wv_sb = [pp.tile([128, GC], bf16, name=f"wv{t}", tag=f"wv{t}") for t in range(2)]
        wout_sb = pp.tile([128, 256], bf16, name="wout", tag="wout")
        bq_sb = pp.tile([128, 1], f32, name="bq", tag="bq")
        bk_sb = pp.tile([128, 1], f32, name="bk", tag="bk")
        QT = pp.tile([128, S], bf16, name="QT", tag="QT")
        KTt = pp.tile([128, S], bf16, name="KT", tag="KT")
        Vt = [pp.tile([128, GC], bf16, name=f"V{i}", tag=f"V{i}") for i in range(NKT)]
        ones_f = pp.tile([128, 32], f32, name="onesf", tag="onesf")
        ones_b = pp.tile([128, 32], bf16, name="onesb", tag="onesb")
        zc = pp.tile([128, 32], f32, name="zc", tag="zc")

        # the persistent 4-bank score tile: head h owns bank h (512 cols;
        # two 256-col halves, consecutive groups alternate halves — matmul
        # dsts must not cross a PSUM bank boundary)
        st = stps.tile([128, 2048], f32, name="st", tag="st")
        st_h = st[:].rearrange("p (h c) -> p h c", h=4)  # [128, 4, 512]

        # ones via exp(0): exact 1.0 and pre-loads the ACT exp table early
        nc.vector.memset(zc[:], 0.0)
        nc.scalar.activation(ones_f[:], zc[:], Exp)
        nc.vector.tensor_copy(ones_b[:], ones_f[:])

        # --- input DMA (bf16 direct); first 320 x-cols land first so
        # chunk 0's QKV/scores start early ---
        x1v = x1_d[:].rearrange("(t p) f -> t p f", p=128)
        x2v = x2_d[:].rearrange("(t p) f -> t p f", p=128)
        wqkv = wqk_d[:].rearrange("(t p) f -> t p f", p=128)
        wvv = wv_d[:].rearrange("(t p) f -> t p f", p=128)
        # critical path first: weights + the x columns chunk 0 needs
        # (Q c0 / K c0 contract x1[:, 0:256] on both C-halves), so the
        # first scores/exp start ~3us in instead of queueing behind the
        # ~1MB bulk x transfer.
        for t in range(2):
            nc.sync.dma_start(wqk_sb[t][:], wqkv[t])
        nc.sync.dma_start(bq_sb[:], bqk_d[0])
        nc.sync.dma_start(bk_sb[:], bqk_d[1])
        for t in range(2):
            nc.sync.dma_start(xT[t][:, 0:512], x1v[t][:, 0:512])
        for t in range(2):
            nc.sync.dma_start(wv_sb[t][:], wvv[t])
            nc.sync.dma_start(xT[t][:, 512:N], x1v[t][:, 512:N])
        for t in range(2):
            nc.sync.dma_start(xT[t][:, N:N + 512], x2v[t][:, 0:512])
            nc.sync.dma_start(xT[t][:, N + 512:S], x2v[t][:, 512:N])
        nc.sync.dma_start(wout_sb[:], wout_d[:])

        # ---- emit helpers ----
        def ut_tile(nm):
            # [128, 512] f32 = 1 bank; pool has 2 bufs
            return utps.tile([128, 512], f32, name=nm, tag="ut")

        def emit_qk(c0, c1, m, bias_t, out_t, nm):
            w = c1 - c0
            ps = ut_tile(nm)
            for t in range(2):
                nc.tensor.matmul(
                    ps[:, :w], wqk_sb[t][:, 128 * m:128 * m + 128], xT[t][:, c0:c1],
                    start=(t == 0), stop=(t == 1),
                )
            nc.vector.tensor_scalar_add(out_t[:, c0:c1], ps[:, :w], bias_t[:])

        def emit_v(i, nm):
            o, sz = KTILES[i]
            ps = ut_tile(nm)
            for t in range(2):
                nc.tensor.matmul(
                    ps[:sz, 0:GC], xT[t][:, o:o + sz], wv_sb[t][:],
                    start=(t == 0), stop=(t == 1),
                )
            nc.vector.tensor_copy(Vt[i][:sz, :], ps[:sz, 0:GC])

        def emit_scores(c0, w, i, half, off=0):
            # head h -> its own PSUM bank (concurrent row-packed matmuls
            # must not share a bank). No half alternation: the conservative
            # bbox dependency serializes scores<->exp anyway, so each head
            # uses its full 512-col bank and groups are 2x wider (fewer
            # chain links, fewer per-instr overheads).
            o, sz = KTILES[i]
            for h in range(4):
                base = 512 * h + off
                nc.tensor.matmul(
                    st[:sz, base:base + w],
                    KTt[32 * h:32 * h + 32, o:o + sz],
                    QT[32 * h:32 * h + 32, c0:c0 + w],
                    start=True, stop=True,
                    tile_position=(32 * h, 0),
                )

        def emit_exp(w, sz, half, P, nm, off=0):
            stv = st_h[:sz, :, off:off + w]
            if w == PW:
                nc.scalar.activation(P[:sz, :], st[:sz, :], Exp)
            else:
                pv = P[:sz, :].rearrange("p (h c) -> p h c", h=4)[:, :, 0:w]
                nc.scalar.activation(pv, stv, Exp)

        def emit_av(P, w, i, attnT_ps, start, stop):
            o, sz = KTILES[i]
            for h in range(4):
                nc.tensor.matmul(
                    attnT_ps[32 * h:32 * h + 32, :w],
                    Vt[i][:sz, 32 * h:32 * h + 32],
                    P[:sz, PW * h:PW * h + w],
                    start=start, stop=stop, skip_group_check=True,
                    tile_position=(0, 32 * h),
                )

        def emit_den(P, w, i, den_ps, start, stop):
            o, sz = KTILES[i]
            for h in range(4):
                nc.tensor.matmul(
                    den_ps[32 * h:32 * h + 32, 0:w],
                    ones_b[:sz, :],
                    P[:sz, PW * h:PW * h + w],
                    start=start, stop=stop, skip_group_check=True,
                    tile_position=(0, 32 * h),
                )

        def emit_tail(c0, w, attnT_ps, den_ps, nm):
            # den is broadcast across each head's 32 partitions (ones lhsT
            # is [sz, 32]), so normalization is reciprocal + multiply only.
            recip_f = sb.tile([128, PW], f32, name=f"rf{nm}", tag="recipf")
            nc.vector.reciprocal_approx_fast(recip_f[:, :w], den_ps[:, :w])
            attn_sb = sb.tile([128, PW], bf16, name=f"at{nm}", tag="attnsb")
            nc.vector.tensor_mul(attn_sb[:, :w], attnT_ps[:, :w], recip_f[:, :w])
            nsub = (w + 127) // 128
            for s4 in range(nsub):
                ssz = min(128, w - 128 * s4)
                off = 128 * s4
                yp = ut_tile(f"yp{nm}{s4}")
                nc.tensor.matmul(
                    yp[:ssz, 0:256], attn_sb[:, off:off + ssz], wout_sb[:],
                    start=True, stop=True,
                )
                ysb = sb.tile([128, 256], f32, name=f"ys{nm}{s4}", tag="ysb")
                nc.vector.tensor_copy(ysb[:ssz, :], yp[:ssz, 0:256])
                nc.sync.dma_start(y_d[c0 + off:c0 + off + ssz, :], ysb[:ssz, :])

        # ---- deferred production schedule ----
        # K chunk j covers score k-tiles 2j, 2j+1 -> keep 3-4 tiles ahead.
        # V k-tile j is consumed by group (0, j) -> produce at (0, j-2).
        # Q chunk c+1 is consumed from chunk c+1 -> produce mid-chunk c.
        pending = {}

        def defer(c, i, th):
            pending.setdefault((c, i), []).append(th)

        # K chunk j covers tokens [c0, c0+w); first score k-tile touching
        # it is floor(c0/128) — emit 2-3 k-tiles ahead.
        for j in range(1, NCH):
            c0j, wj = CHUNKS[j]
            ii = max(0, c0j // 128 - 3)
            defer(0, ii, lambda c0j=c0j, wj=wj, j=j: emit_qk(
                c0j, c0j + wj, 1, bk_sb, KTt, f"k{j}"))
        for j in range(2, NKT):
            defer(0, j - 2, lambda j=j: emit_v(j, f"v{j}"))
        for c in range(NCH - 1):
            defer(c, 6, lambda c=c: emit_qk(
                CHUNKS[c + 1][0], CHUNKS[c + 1][0] + CHUNKS[c + 1][1],
                0, bq_sb, QT, f"q{c+1}"))

        # ---- prologue ----
        emit_qk(0, 512, 0, bq_sb, QT, "q0")
        emit_qk(0, 512, 1, bk_sb, KTt, "k0")
        emit_v(0, "v0")
        emit_v(1, "v1")

        # ---- main pipeline over NCH chunks x NKT ktiles ----
        groups = [(c, i) for c in range(3) for i in range(NKT)]
        emit_scores(0, CHUNKS[0][1], 0, 0)
        attnT_ps = den_ps = None
        tail_thunk = None
        avden_prev = None
        for g, (c, i) in enumerate(groups):
            cw0, cww = CHUNKS[c]
            if i == 0:
                attnT_ps = avps.tile([128, PW], f32, name=f"attnT{c}", tag="attnT",
                                     padded_shape=[128, 512])
                den_ps = dnps.tile([128, PW], f32, name=f"den{c}", tag="den",
                                   padded_shape=[128, 512])
            o, sz = KTILES[i]
            P = psb.tile([128, 4 * PW], bf16, name=f"P{c}_{i}", tag="P")
            # Without half alternation scores(g+1) overwrite the slot
            # exp(g) reads, so exp MUST be emitted first (the tracker pairs
            # each read with the last prior write). The resulting chain
            # exp(g) -> scores(g+1) -> exp(g+1) is the same serialization
            # the bbox edges forced anyway; wider groups amortize it.
            emit_exp(cww, sz, g % 2, P, f"e{c}_{i}")
            if g + 1 < len(groups):
                nc2, ni = groups[g + 1]
                emit_scores(CHUNKS[nc2][0], CHUNKS[nc2][1], ni, (g + 1) % 2)
            # tail of chunk c-1 precedes this chunk's first AV (avps/dnps
            # have one buf; the pool WAR edge needs mul already emitted)
            if tail_thunk is not None and i == 1:
                tail_thunk()
                tail_thunk = None
            # deferred by one group so they never gate the next scores:
            # AV/den of group g-1 run on PE while ACT exps group g
            if avden_prev is not None:
                avden_prev()
                avden_prev = None
            avden_prev = (lambda P=P, w=cww, i=i, a=attnT_ps, d=den_ps:
                          (emit_av(P, w, i, a, start=(i == 0), stop=(i == NKT - 1)),
                           emit_den(P, w, i, d, start=(i == 0), stop=(i == NKT - 1))))
            for th in pending.get((c, i), ()):
                th()
            if i == NKT - 1:
                tail_thunk = (lambda c0=cw0, w=cww, a=attnT_ps, d=den_ps, c=c:
                              emit_tail(c0, w, a, d, f"t{c}"))
        avden_prev()
        avden_prev = None

        # ---- rump chunk (q=1536:1568, w=32): all 13 k-tiles packed into
        # ONE score/exp group (k-tile j at cols 512h+32j), one chain link
        # (~4us) instead of 13 serialized tiny groups (~11us) ----
        for j in range(NKT):
            emit_scores(1536, 32, j, 0, off=32 * j)
        PR = psb.tile([128, 4 * PW], bf16, name="PR", tag="P")
        stvR = st_h[:, :, 0:32 * NKT]
        pvR = PR[:, :].rearrange("p (h c) -> p h c", h=4)[:, :, 0:32 * NKT]
        nc.scalar.activation(pvR, stvR, Exp)
        if tail_thunk is not None:
            tail_thunk()
            tail_thunk = None
        attnT_ps = avps.tile([128, PW], f32, name="attnTr", tag="attnT",
                             padded_shape=[128, 512])
        den_ps = dnps.tile([128, PW], f32, name="denr", tag="den",
                           padded_shape=[128, 512])
        for j in range(NKT):
            o, sz = KTILES[j]
            for h in range(4):
                nc.tensor.matmul(
                    attnT_ps[32 * h:32 * h + 32, 0:32],
                    Vt[j][:sz, 32 * h:32 * h + 32],
                    PR[:sz, 512 * h + 32 * j:512 * h + 32 * j + 32],
                    start=(j == 0), stop=(j == NKT - 1), skip_group_check=True,
                    tile_position=(0, 32 * h),
                )
            for h in range(4):
                nc.tensor.matmul(
                    den_ps[32 * h:32 * h + 32, 0:32],
                    ones_b[:sz, :],
                    PR[:sz, 512 * h + 32 * j:512 * h + 32 * j + 32],
                    start=(j == 0), stop=(j == NKT - 1), skip_group_check=True,
                    tile_position=(0, 32 * h),
                )
        emit_tail(1536, 32, attnT_ps, den_ps, "tr")

        ctx.close()

    nc.compile()
    return nc


def prepare_in_maps(x1, x2, pos_emb, w_qkv, b_qkv, w_out, b_out):
    import ml_dtypes

    bf16 = ml_dtypes.bfloat16
    x1 = np.asarray(x1, dtype=np.float32)
    x2 = np.asarray(x2, dtype=np.float32)
    pos = np.asarray(pos_emb, dtype=np.float32).reshape(C)
    w_qkv = np.asarray(w_qkv, dtype=np.float32)
    b_qkv = np.asarray(b_qkv, dtype=np.float32)
    w_out = np.asarray(w_out, dtype=np.float32)
    b_out = np.asarray(b_out, dtype=np.float32)

    scale = 1.0 / np.sqrt(np.float32(DH))
    b_eff = b_qkv + w_qkv @ pos
    wq = w_qkv[0:C] * scale
    bq = b_eff[0:C] * scale
    wk = w_qkv[C:2 * C]
    bk = b_eff[C:2 * C]
    wv = w_qkv[2 * C:3 * C]
    bv = b_eff[2 * C:3 * C]

    in_maps = []
    for core in range(8):
        b = core // 2
        g = core % 2
        gsl = slice(GC * g, GC * (g + 1))
        wqkT = np.concatenate([wq[gsl], wk[gsl]], axis=0).T.copy()     # [C, 256]
        wvT = wv[gsl].T.copy()                                         # [C, GC]
        woutT = w_out[:, gsl].T.copy()                                 # [GC, 256]
        bqk = np.stack([bq[gsl], bk[gsl]])[:, :, None].copy()          # [2, 128, 1]
        in_maps.append({
            "x1b": np.ascontiguousarray(x1[b].reshape(C, N)).astype(bf16),
            "x2b": np.ascontiguousarray(x2[b].reshape(C, N)).astype(bf16),
            "wqkT": np.ascontiguousarray(wqkT).astype(bf16),
            "wvT": np.ascontiguousarray(wvT).astype(bf16),
            "woutT": np.ascontiguousarray(woutT).astype(bf16),
            "bqk": np.ascontiguousarray(bqk),
        })
    # out1+out2 folds two tokens, each carrying b_out and the V-bias term
    y_const = 2.0 * (b_out + w_out @ bv)  # [C]
    return in_maps, y_const


def get_nc(repeat=1):
    key = repeat
    if key not in _cache:
        _cache[key] = _build_nc(repeat)
    return _cache[key]


def assemble(per_core_y, y_const):
    out = np.empty((B, C, H, W), dtype=np.float32)
    for b in range(B):
        yb = per_core_y[2 * b] + per_core_y[2 * b + 1]                 # [S, C]
        yf = yb[:N] + yb[N:] + y_const[None, :]                        # [N, C]
        out[b] = yf.T.reshape(C, H, W)
    return out


def kernel(x1, x2, pos_emb, w_qkv, b_qkv, w_out, b_out):
    global LAST_RESULTS, LAST_IN_MAPS
    from concourse.bass_utils import run_bass_kernel_spmd

    in_maps, y_const = prepare_in_maps(x1, x2, pos_emb, w_qkv, b_qkv, w_out, b_out)
    LAST_IN_MAPS = in_maps
    nc = get_nc()
    res = run_bass_kernel_spmd(nc, in_maps, core_ids=list(range(8)))
    LAST_RESULTS = res
    return assemble([res.results[c]["y"] for c in range(8)], y_const)


# revision 38
# speedup vs baseline: 19585.4111x; 1.0157x over previous
"""Trainium2 Bass kernel for EnhancedCrossAttention.

Shapes (hardcoded): B=4, C=256, H=W=28, heads=8, head_dim=32.
Sharding: 8 cores = 4 batches x 2 head-groups (4 heads each core).
Each core computes its batch's QKV (its head-group's Q/K/V), attention for
4 heads, and a partial out-projection (contracting its 128 attention-output
channels). Host sums the two partials per batch, adds the folded bias,
folds the two spatial halves, and reshapes.

Host-side algebraic folds (all exact):
  - pos_emb enters only via the QKV matmul: b_eff = b_qkv + w_qkv @ pos
  - 1/sqrt(dh) folded into Q weights+bias
  - V bias contributes attn_out += b_v (softmax weights sum to 1), folded
    through w_out into a constant added on the host.

v2 design (vs v1): the v1 trace showed PE and ACT strictly alternating at a
~3.9us period per k-tile — the in-order PE queue was [scores_i,
AV_i(waits exp_i), scores_{i+1}], so scores_{i+1} could not run during
exp_i even though its inputs were ready. v2 software-pipelines: the scores
for k-tile group g+1 are emitted into the PE stream BEFORE the AV/den of
group g, so the PE works through them while ACT exps group g.

Hardware constraint (found by bisection): concurrent row-quadrant-packed
matmuls (tile_position=(32h, 0)) must NOT write the same PSUM bank — two
heads' score matmuls writing the same partitions of one bank crash the
exec unit (NRT_EXEC_UNIT_UNRECOVERABLE). So scores use ONE persistent
[128, 2048] tile where head h owns bank h (512 f32 cols), and chunks are
256 q-tokens: group g writes the 256-col half (g%2) of each head's bank.
The half alternation gives double-buffering within 4 banks; attnT (1) +
den (1) + a 2-buf utility pool (2) for QKV/V/out-proj = 8 banks exactly.
All matmul operands are bf16 (1 col/cycle; fp32 is 4) and are cast
host-side so no on-chip round-copy passes are needed. den's stationary
ones operand is [sz, 32], which lands the denominator broadcast across
each head's 32 partitions at identical stream cost, so normalization is
just reciprocal+multiply on DVE (no PE broadcast matmul). Q/K/V
production is deferred into the pipeline through the utility pool so the
exp stream starts ~2us in and QKV hides under early exps.
"""

import numpy as np

B, C, H, W = 4, 256, 28, 28
N = H * W            # 784
S = 2 * N            # 1568 tokens
NH = 8
DH = 32
GH = 4               # heads per group (per core)
GC = GH * DH         # 128 channels per group

PW = 512             # head slot width in the score tile (full bank)
CHUNKS = [(0, 512), (512, 512), (1024, 512), (1536, 32)]
NCH = len(CHUNKS)    # 4 chunks covering 1568 q-tokens exactly
KTILES = [(i * 128, min(128, S - i * 128)) for i in range((S + 127) // 128)]
NKT = len(KTILES)    # 13 (12 full + one 32-row k-rump)

_cache = {}
LAST_RESULTS = None
LAST_IN_MAPS = None


def _build_nc(repeat=1):
    import concourse.mybir as mybir
    import concourse.tile as tile
    from concourse import bacc
    from contextlib import ExitStack

    f32 = mybir.dt.float32
    bf16 = mybir.dt.bfloat16
    Exp = mybir.ActivationFunctionType.Exp

    nc = bacc.Bacc("TRN2", target_bir_lowering=False, debug=False)

    x1_d = nc.dram_tensor("x1b", [C, N], bf16, kind="ExternalInput")
    x2_d = nc.dram_tensor("x2b", [C, N], bf16, kind="ExternalInput")
    wqk_d = nc.dram_tensor("wqkT", [C, 256], bf16, kind="ExternalInput")
    wv_d = nc.dram_tensor("wvT", [C, GC], bf16, kind="ExternalInput")
    wout_d = nc.dram_tensor("woutT", [GC, 256], bf16, kind="ExternalInput")
    bqk_d = nc.dram_tensor("bqk", [2, 128, 1], f32, kind="ExternalInput")
    y_d = nc.dram_tensor("y", [S, C], f32, kind="ExternalOutput")

    with tile.TileContext(nc) as tc:
      for _rep in range(repeat):
        ctx = ExitStack()
        pp = ctx.enter_context(tc.tile_pool(name="persist", bufs=1))
        sb = ctx.enter_context(tc.tile_pool(name="work", bufs=3))
        psb = ctx.enter_context(tc.tile_pool(name="pwork", bufs=3))
        stps = ctx.enter_context(tc.tile_pool(name="stps", bufs=1, space="PSUM"))
        avps = ctx.enter_context(tc.tile_pool(name="avps", bufs=1, space="PSUM"))
        dnps = ctx.enter_context(tc.tile_pool(name="dnps", bufs=1, space="PSUM"))
        utps = ctx.enter_context(tc.tile_pool(name="utps", bufs=2, space="PSUM"))

        xT = [pp.tile([128, S], bf16, name=f"xT{t}", tag=f"xT{t}") for t in range(2)]
        wqk_sb = [pp.tile([128, 256], bf16, name=f"wqk{t}", tag=f"wqk{t}") for t in range(2)]
        wv_sb = [pp.tile([128, GC], bf16, name=f"wv{t}", tag=f"wv{t}") for t in range(2)]
        wout_sb = pp.tile([128, 256], bf16, name="wout", tag="wout")
        bq_sb = pp.tile([128, 1], f32, name="bq", tag="bq")
        bk_sb = pp.tile([128, 1], f32, name="bk", tag="bk")
        QT = pp.tile([128, S], bf16, name="QT", tag="QT")
        KTt = pp.tile([128, S], bf16, name="KT", tag="KT")
        Vt = [pp.tile([128, GC], bf16, name=f"V{i}", tag=f"V{i}") for i in range(NKT)]
        ones_f = pp.tile([128, 32], f32, name="onesf", tag="onesf")
        ones_b = pp.tile([128, 32], bf16, name="onesb", tag="onesb")
        zc = pp.tile([128, 32], f32, name="zc", tag="zc")

        # the persistent 4-bank score tile: head h owns bank h (512 cols;
        # two 256-col halves, consecutive groups alternate halves — matmul
        # dsts must not cross a PSUM bank boundary)
        st = stps.tile([128, 2048], f32, name="st", tag="st")
        st_h = st[:].rearrange("p (h c) -> p h c", h=4)  # [128, 4, 512]

        # ones via exp(0): exact 1.0 and pre-loads the ACT exp table early
        nc.vector.memset(zc[:], 0.0)
        nc.scalar.activation(ones_f[:], zc[:], Exp)
        nc.vector.tensor_copy(ones_b[:], ones_f[:])

        # --- input DMA (bf16 direct); first 320 x-cols land first so
        # chunk 0's QKV/scores start early ---
        x1v = x1_d[:].rearrange("(t p) f -> t p f", p=128)
        x2v = x2_d[:].rearrange("(t p) f -> t p f", p=128)
        wqkv = wqk_d[:].rearrange("(t p) f -> t p f", p=128)
        wvv = wv_d[:].rearrange("(t p) f -> t p f", p=128)
        # critical path first: weights + the x columns chunk 0 needs
        # (Q c0 / K c0 contract x1[:, 0:256] on both C-halves), so the
        # first scores/exp start ~3us in instead of queueing behind the
        # ~1MB bulk x transfer.
        for t in range(2):
            nc.sync.dma_start(wqk_sb[t][:], wqkv[t])
        for t in range(2):
            nc.sync.dma_start(xT[t][:, 0:512], x1v[t][:, 0:512])
        nc.sync.dma_start(bq_sb[:], bqk_d[0])
        nc.sync.dma_start(bk_sb[:], bqk_d[1])
        for t in range(2):
            nc.sync.dma_start(wv_sb[t][:], wvv[t])
            nc.sync.dma_start(xT[t][:, 512:N], x1v[t][:, 512:N])
        for t in range(2):
            nc.sync.dma_start(xT[t][:, N:N + 512], x2v[t][:, 0:512])
            nc.sync.dma_start(xT[t][:, N + 512:S], x2v[t][:, 512:N])
        nc.sync.dma_start(wout_sb[:], wout_d[:])

        # ---- emit helpers ----
        def ut_tile(nm):
            # [128, 512] f32 = 1 bank; pool has 2 bufs
            return utps.tile([128, 512], f32, name=nm, tag="ut")

        def emit_qk(c0, c1, m, bias_t, out_t, nm):
            w = c1 - c0
            ps = ut_tile(nm)
            for t in range(2):
                nc.tensor.matmul(
                    ps[:, :w], wqk_sb[t][:, 128 * m:128 * m + 128], xT[t][:, c0:c1],
                    start=(t == 0), stop=(t == 1),
                )
            nc.vector.tensor_scalar_add(out_t[:, c0:c1], ps[:, :w], bias_t[:])

        def emit_v(i, nm):
            o, sz = KTILES[i]
            ps = ut_tile(nm)
            for t in range(2):
                nc.tensor.matmul(
                    ps[:sz, 0:GC], xT[t][:, o:o + sz], wv_sb[t][:],
                    start=(t == 0), stop=(t == 1),
                )
            nc.vector.tensor_copy(Vt[i][:sz, :], ps[:sz, 0:GC])

        def emit_scores(c0, w, i, half, off=0):
            # head h -> its own PSUM bank (concurrent row-packed matmuls
            # must not share a bank). No half alternation: the conservative
            # bbox dependency serializes scores<->exp anyway, so each head
            # uses its full 512-col bank and groups are 2x wider (fewer
            # chain links, fewer per-instr overheads).
            o, sz = KTILES[i]
            for h in range(4):
                base = 512 * h + off
                nc.tensor.matmul(
                    st[:sz, base:base + w],
                    KTt[32 * h:32 * h + 32, o:o + sz],
                    QT[32 * h:32 * h + 32, c0:c0 + w],
                    start=True, stop=True,
                    tile_position=(32 * h, 0),
                )

        def emit_exp(w, sz, half, P, nm, off=0):
            stv = st_h[:sz, :, off:off + w]
            if w == PW:
                nc.scalar.activation(P[:sz, :], st[:sz, :], Exp)
            else:
                pv = P[:sz, :].rearrange("p (h c) -> p h c", h=4)[:, :, 0:w]
                nc.scalar.activation(pv, stv, Exp)

        def emit_av(P, w, i, attnT_ps, start, stop):
            o, sz = KTILES[i]
            for h in range(4):
                nc.tensor.matmul(
                    attnT_ps[32 * h:32 * h + 32, :w],
                    Vt[i][:sz, 32 * h:32 * h + 32],
                    P[:sz, PW * h:PW * h + w],
                    start=start, stop=stop, skip_group_check=True,
                    tile_position=(0, 32 * h),
                )

        def emit_den(P, w, i, den_ps, start, stop):
            o, sz = KTILES[i]
            for h in range(4):
                nc.tensor.matmul(
                    den_ps[32 * h:32 * h + 32, 0:w],
                    ones_b[:sz, :],
                    P[:sz, PW * h:PW * h + w],
                    start=start, stop=stop, skip_group_check=True,
                    tile_position=(0, 32 * h),
                )

        def emit_tail(c0, w, attnT_ps, den_ps, nm):
            # den is broadcast across each head's 32 partitions (ones lhsT
            # is [sz, 32]), so normalization is reciprocal + multiply only.
            recip_f = sb.tile([128, PW], f32, name=f"rf{nm}", tag="recipf")
            nc.vector.reciprocal_approx_fast(recip_f[:, :w], den_ps[:, :w])
            attn_sb = sb.tile([128, PW], bf16, name=f"at{nm}", tag="attnsb")
            nc.vector.tensor_mul(attn_sb[:, :w], attnT_ps[:, :w], recip_f[:, :w])
            nsub = (w + 127) // 128
            for s4 in range(nsub):
                ssz = min(128, w - 128 * s4)
                off = 128 * s4
                yp = ut_tile(f"yp{nm}{s4}")
                nc.tensor.matmul(
                    yp[:ssz, 0:256], attn_sb[:, off:off + ssz], wout_sb[:],
                    start=True, stop=True,
                )
                ysb = sb.tile([128, 256], f32, name=f"ys{nm}{s4}", tag="ysb")
                nc.vector.tensor_copy(ysb[:ssz, :], yp[:ssz, 0:256])
                nc.sync.dma_start(y_d[c0 + off:c0 + off + ssz, :], ysb[:ssz, :])

        # ---- deferred production schedule ----
        # K chunk j covers score k-tiles 2j, 2j+1 -> keep 3-4 tiles ahead.
        # V k-tile j is consumed by group (0, j) -> produce at (0, j-2).
        # Q chunk c+1 is consumed from chunk c+1 -> produce mid-chunk c.
        pending = {}

        def defer(c, i, th):
            pending.setdefault((c, i), []).append(th)

        # K chunk j covers tokens [c0, c0+w); first score k-tile touching
        # it is floor(c0/128) — emit 2-3 k-tiles ahead.
        for j in range(1, NCH):
            c0j, wj = CHUNKS[j]
            ii = max(0, c0j // 128 - 3)
            defer(0, ii, lambda c0j=c0j, wj=wj, j=j: emit_qk(
                c0j, c0j + wj, 1, bk_sb, KTt, f"k{j}"))
        for j in range(2, NKT):
            defer(0, j - 2, lambda j=j: emit_v(j, f"v{j}"))
        for c in range(NCH - 1):
            defer(c, 6, lambda c=c: emit_qk(
                CHUNKS[c + 1][0], CHUNKS[c + 1][0] + CHUNKS[c + 1][1],
                0, bq_sb, QT, f"q{c+1}"))

        # ---- prologue ----
        emit_qk(0, 512, 0, bq_sb, QT, "q0")
        emit_qk(0, 512, 1, bk_sb, KTt, "k0")
        emit_v(0, "v0")
        emit_v(1, "v1")

        # ---- main pipeline over NCH chunks x NKT ktiles ----
        groups = [(c, i) for c in range(3) for i in range(NKT)]
        emit_scores(0, CHUNKS[0][1], 0, 0)
        attnT_ps = den_ps = None
        tail_thunk = None
        avden_prev = None
        for g, (c, i) in enumerate(groups):
            cw0, cww = CHUNKS[c]
            if i == 0:
                attnT_ps = avps.tile([128, PW], f32, name=f"attnT{c}", tag="attnT",
                                     padded_shape=[128, 512])
                den_ps = dnps.tile([128, PW], f32, name=f"den{c}", tag="den",
                                   padded_shape=[128, 512])
            o, sz = KTILES[i]
            P = psb.tile([128, 4 * PW], bf16, name=f"P{c}_{i}", tag="P")
            # Without half alternation scores(g+1) overwrite the slot
            # exp(g) reads, so exp MUST be emitted first (the tracker pairs
            # each read with the last prior write). The resulting chain
            # exp(g) -> scores(g+1) -> exp(g+1) is the same serialization
            # the bbox edges forced anyway; wider groups amortize it.
            emit_exp(cww, sz, g % 2, P, f"e{c}_{i}")
            if g + 1 < len(groups):
                nc2, ni = groups[g + 1]
                emit_scores(CHUNKS[nc2][0], CHUNKS[nc2][1], ni, (g + 1) % 2)
            # tail of chunk c-1 precedes this chunk's first AV (avps/dnps
            # have one buf; the pool WAR edge needs mul already emitted)
            if tail_thunk is not None and i == 1:
                tail_thunk()
                tail_thunk = None
            # deferred by one group so they never gate the next scores:
            # AV/den of group g-1 run on PE while ACT exps group g
            if avden_prev is not None:
                avden_prev()
                avden_prev = None
            avden_prev = (lambda P=P, w=cww, i=i, a=attnT_ps, d=den_ps:
                          (emit_av(P, w, i, a, start=(i == 0), stop=(i == NKT - 1)),
                           emit_den(P, w, i, d, start=(i == 0), stop=(i == NKT - 1))))
            for th in pending.get((c, i), ()):
                th()
            if i == NKT - 1:
                tail_thunk = (lambda c0=cw0, w=cww, a=attnT_ps, d=den_ps, c=c:
                              emit_tail(c0, w, a, d, f"t{c}"))
        avden_prev()
        avden_prev = None

        # ---- rump chunk (q=1536:1568, w=32): all 13 k-tiles packed into
        # ONE score/exp group (k-tile j at cols 512h+32j), one chain link
        # (~4us) instead of 13 serialized tiny groups (~11us) ----
        for j in range(NKT):
            emit_scores(1536, 32, j, 0, off=32 * j)
        PR = psb.tile([128, 4 * PW], bf16, name="PR", tag="P")
        # two exp halves: AV/den of k-tiles 0-5 run under exp of 6-12
        stvA = st_h[:, :, 0:192]
        pvA = PR[:, :].rearrange("p (h c) -> p h c", h=4)[:, :, 0:192]
        nc.scalar.activation(pvA, stvA, Exp)
        stvB = st_h[:, :, 192:32 * NKT]
        pvB = PR[:, :].rearrange("p (h c) -> p h c", h=4)[:, :, 192:32 * NKT]
        nc.scalar.activation(pvB, stvB, Exp)
        if tail_thunk is not None:
            tail_thunk()
            tail_thunk = None
        attnT_ps = avps.tile([128, PW], f32, name="attnTr", tag="attnT",
                             padded_shape=[128, 512])
        den_ps = dnps.tile([128, PW], f32, name="denr", tag="den",
                           padded_shape=[128, 512])
        for j in range(NKT):
            o, sz = KTILES[j]
            for h in range(4):
                nc.tensor.matmul(
                    attnT_ps[32 * h:32 * h + 32, 0:32],
                    Vt[j][:sz, 32 * h:32 * h + 32],
                    PR[:sz, 512 * h + 32 * j:512 * h + 32 * j + 32],
                    start=(j == 0), stop=(j == NKT - 1), skip_group_check=True,
                    tile_position=(0, 32 * h),
                )
            for h in range(4):
                nc.tensor.matmul(
                    den_ps[32 * h:32 * h + 32, 0:32],
                    ones_b[:sz, :],
                    PR[:sz, 512 * h + 32 * j:512 * h + 32 * j + 32],
                    start=(j == 0), stop=(j == NKT - 1), skip_group_check=True,
                    tile_position=(0, 32 * h),
                )
        emit_tail(1536, 32, attnT_ps, den_ps, "tr")

        ctx.close()

    nc.compile()
    return nc


def prepare_in_maps(x1, x2, pos_emb, w_qkv, b_qkv, w_out, b_out):
    import ml_dtypes

    bf16 = ml_dtypes.bfloat16
    x1 = np.asarray(x1, dtype=np.float32)
    x2 = np.asarray(x2, dtype=np.float32)
    pos = np.asarray(pos_emb, dtype=np.float32).reshape(C)
    w_qkv = np.asarray(w_qkv, dtype=np.float32)
    b_qkv = np.asarray(b_qkv, dtype=np.float32)
    w_out = np.asarray(w_out, dtype=np.float32)
    b_out = np.asarray(b_out, dtype=np.float32)

    scale = 1.0 / np.sqrt(np.float32(DH))
    b_eff = b_qkv + w_qkv @ pos
    wq = w_qkv[0:C] * scale
    bq = b_eff[0:C] * scale
    wk = w_qkv[C:2 * C]
    bk = b_eff[C:2 * C]
    wv = w_qkv[2 * C:3 * C]
    bv = b_eff[2 * C:3 * C]

    in_maps = []
    for core in range(8):
        b = core // 2
        g = core % 2
        gsl = slice(GC * g, GC * (g + 1))
        wqkT = np.concatenate([wq[gsl], wk[gsl]], axis=0).T.copy()     # [C, 256]
        wvT = wv[gsl].T.copy()                                         # [C, GC]
        woutT = w_out[:, gsl].T.copy()                                 # [GC, 256]
        bqk = np.stack([bq[gsl], bk[gsl]])[:, :, None].copy()          # [2, 128, 1]
        in_maps.append({
            "x1b": np.ascontiguousarray(x1[b].reshape(C, N)).astype(bf16),
            "x2b": np.ascontiguousarray(x2[b].reshape(C, N)).astype(bf16),
            "wqkT": np.ascontiguousarray(wqkT).astype(bf16),
            "wvT": np.ascontiguousarray(wvT).astype(bf16),
            "woutT": np.ascontiguousarray(woutT).astype(bf16),
            "bqk": np.ascontiguousarray(bqk),
        })
    # out1+out2 folds two tokens, each carrying b_out and the V-bias term
    y_const = 2.0 * (b_out + w_out @ bv)  # [C]
    return in_maps, y_const


def get_nc(repeat=1):
    key = repeat
    if key not in _cache:
        _cache[key] = _build_nc(repeat)
    return _cache[key]


def assemble(per_core_y, y_const):
    out = np.empty((B, C, H, W), dtype=np.float32)
    for b in range(B):
        yb = per_core_y[2 * b] + per_core_y[2 * b + 1]                 # [S, C]
        yf = yb[:N] + yb[N:] + y_const[None, :]                        # [N, C]
        out[b] = yf.T.reshape(C, H, W)
    return out


def kernel(x1, x2, pos_emb, w_qkv, b_qkv, w_out, b_out):
    global LAST_RESULTS, LAST_IN_MAPS
    from concourse.bass_utils import run_bass_kernel_spmd

    in_maps, y_const = prepare_in_maps(x1, x2, pos_emb, w_qkv, b_qkv, w_out, b_out)
    LAST_IN_MAPS = in_maps
    nc = get_nc()
    res = run_bass_kernel_spmd(nc, in_maps, core_ids=list(range(8)))
    LAST_RESULTS = res
    return assemble([res.results[c]["y"] for c in range(8)], y_const)


# revision 39
# speedup vs baseline: 19969.5508x; 1.0196x over previous
"""Trainium2 Bass kernel for EnhancedCrossAttention.

Shapes (hardcoded): B=4, C=256, H=W=28, heads=8, head_dim=32.
Sharding: 8 cores = 4 batches x 2 head-groups (4 heads each core).
Each core computes its batch's QKV (its head-group's Q/K/V), attention for
4 heads, and a partial out-projection (contracting its 128 attention-output
channels). Host sums the two partials per batch, adds the folded bias,
folds the two spatial halves, and reshapes.

Host-side algebraic folds (all exact):
  - pos_emb enters only via the QKV matmul: b_eff = b_qkv + w_qkv @ pos
  - 1/sqrt(dh) folded into Q weights+bias
  - V bias contributes attn_out += b_v (softmax weights sum to 1), folded
    through w_out into a constant added on the host.

v2 design (vs v1): the v1 trace showed PE and ACT strictly alternating at a
~3.9us period per k-tile — the in-order PE queue was [scores_i,
AV_i(waits exp_i), scores_{i+1}], so scores_{i+1} could not run during
exp_i even though its inputs were ready. v2 software-pipelines: the scores
for k-tile group g+1 are emitted into the PE stream BEFORE the AV/den of
group g, so the PE works through them while ACT exps group g.

Hardware constraint (found by bisection): concurrent row-quadrant-packed
matmuls (tile_position=(32h, 0)) must NOT write the same PSUM bank — two
heads' score matmuls writing the same partitions of one bank crash the
exec unit (NRT_EXEC_UNIT_UNRECOVERABLE). So scores use ONE persistent
[128, 2048] tile where head h owns bank h (512 f32 cols), and chunks are
256 q-tokens: group g writes the 256-col half (g%2) of each head's bank.
The half alternation gives double-buffering within 4 banks; attnT (1) +
den (1) + a 2-buf utility pool (2) for QKV/V/out-proj = 8 banks exactly.
All matmul operands are bf16 (1 col/cycle; fp32 is 4) and are cast
host-side so no on-chip round-copy passes are needed. den's stationary
ones operand is [sz, 32], which lands the denominator broadcast across
each head's 32 partitions at identical stream cost, so normalization is
just reciprocal+multiply on DVE (no PE broadcast matmul). Q/K/V
production is deferred into the pipeline through the utility pool so the
exp stream starts ~2us in and QKV hides under early exps.
"""

import numpy as np

B, C, H, W = 4, 256, 28, 28
N = H * W            # 784
S = 2 * N            # 1568 tokens
NH = 8
DH = 32
GH = 4               # heads per group (per core)
GC = GH * DH         # 128 channels per group

PW = 512             # head slot width in the score tile (full bank)
CHUNKS = [(0, 512), (512, 512), (1024, 512), (1536, 32)]
NCH = len(CHUNKS)    # 4 chunks covering 1568 q-tokens exactly
KTILES = [(i * 128, min(128, S - i * 128)) for i in range((S + 127) // 128)]
NKT = len(KTILES)    # 13 (12 full + one 32-row k-rump)

_cache = {}
LAST_RESULTS = None
LAST_IN_MAPS = None


def _build_nc(repeat=1):
    import concourse.mybir as mybir
    import concourse.tile as tile
    from concourse import bacc
    from contextlib import ExitStack

    f32 = mybir.dt.float32
    bf16 = mybir.dt.bfloat16
    Exp = mybir.ActivationFunctionType.Exp

    nc = bacc.Bacc("TRN2", target_bir_lowering=False, debug=False)

    x1_d = nc.dram_tensor("x1b", [C, N], bf16, kind="ExternalInput")
    x2_d = nc.dram_tensor("x2b", [C, N], bf16, kind="ExternalInput")
    wqk_d = nc.dram_tensor("wqkT", [C, 256], bf16, kind="ExternalInput")
    wv_d = nc.dram_tensor("wvT", [C, GC], bf16, kind="ExternalInput")
    wout_d = nc.dram_tensor("woutT", [GC, 256], bf16, kind="ExternalInput")
    bqk_d = nc.dram_tensor("bqk", [2, 128, 1], f32, kind="ExternalInput")
    y_d = nc.dram_tensor("y", [S, C], f32, kind="ExternalOutput")

    with tile.TileContext(nc) as tc:
      for _rep in range(repeat):
        ctx = ExitStack()
        pp = ctx.enter_context(tc.tile_pool(name="persist", bufs=1))
        sb = ctx.enter_context(tc.tile_pool(name="work", bufs=3))
        psb = ctx.enter_context(tc.tile_pool(name="pwork", bufs=3))
        stps = ctx.enter_context(tc.tile_pool(name="stps", bufs=1, space="PSUM"))
        avps = ctx.enter_context(tc.tile_pool(name="avps", bufs=1, space="PSUM"))
        dnps = ctx.enter_context(tc.tile_pool(name="dnps", bufs=1, space="PSUM"))
        utps = ctx.enter_context(tc.tile_pool(name="utps", bufs=2, space="PSUM"))

        xT = [pp.tile([128, S], bf16, name=f"xT{t}", tag=f"xT{t}") for t in range(2)]
        wqk_sb = [pp.tile([128, 256], bf16, name=f"wqk{t}", tag=f"wqk{t}") for t in range(2)]
        wv_sb = [pp.tile([128, GC], bf16, name=f"wv{t}", tag=f"wv{t}") for t in range(2)]
        wout_sb = pp.tile([128, 256], bf16, name="wout", tag="wout")
        bq_sb = pp.tile([128, 1], f32, name="bq", tag="bq")
        bk_sb = pp.tile([128, 1], f32, name="bk", tag="bk")
        QT = pp.tile([128, S], bf16, name="QT", tag="QT")
        KTt = pp.tile([128, S], bf16, name="KT", tag="KT")
        Vt = [pp.tile([128, GC], bf16, name=f"V{i}", tag=f"V{i}") for i in range(NKT)]
        ones_f = pp.tile([128, 32], f32, name="onesf", tag="onesf")
        ones_b = pp.tile([128, 32], bf16, name="onesb", tag="onesb")
        zc = pp.tile([128, 32], f32, name="zc", tag="zc")

        # the persistent 4-bank score tile: head h owns bank h (512 cols;
        # two 256-col halves, consecutive groups alternate halves — matmul
        # dsts must not cross a PSUM bank boundary)
        st = stps.tile([128, 2048], f32, name="st", tag="st")
        st_h = st[:].rearrange("p (h c) -> p h c", h=4)  # [128, 4, 512]

        # ones via exp(0): exact 1.0 and pre-loads the ACT exp table early
        nc.vector.memset(zc[:], 0.0)
        nc.scalar.activation(ones_f[:], zc[:], Exp)
        nc.vector.tensor_copy(ones_b[:], ones_f[:])

        # --- input DMA (bf16 direct); first 320 x-cols land first so
        # chunk 0's QKV/scores start early ---
        x1v = x1_d[:].rearrange("(t p) f -> t p f", p=128)
        x2v = x2_d[:].rearrange("(t p) f -> t p f", p=128)
        wqkv = wqk_d[:].rearrange("(t p) f -> t p f", p=128)
        wvv = wv_d[:].rearrange("(t p) f -> t p f", p=128)
        # critical path first: weights + the x columns chunk 0 needs
        # (Q c0 / K c0 contract x1[:, 0:256] on both C-halves), so the
        # first scores/exp start ~3us in instead of queueing behind the
        # ~1MB bulk x transfer.
        for t in range(2):
            nc.sync.dma_start(wqk_sb[t][:], wqkv[t])
        for t in range(2):
            nc.sync.dma_start(xT[t][:, 0:512], x1v[t][:, 0:512])
        nc.sync.dma_start(bq_sb[:], bqk_d[0])
        nc.sync.dma_start(bk_sb[:], bqk_d[1])
        for t in range(2):
            nc.sync.dma_start(wv_sb[t][:], wvv[t])
            nc.sync.dma_start(xT[t][:, 512:N], x1v[t][:, 512:N])
        for t in range(2):
            nc.sync.dma_start(xT[t][:, N:N + 512], x2v[t][:, 0:512])
            nc.sync.dma_start(xT[t][:, N + 512:S], x2v[t][:, 512:N])
        nc.sync.dma_start(wout_sb[:], wout_d[:])

        # ---- emit helpers ----
        def ut_tile(nm):
            # [128, 512] f32 = 1 bank; pool has 2 bufs
            return utps.tile([128, 512], f32, name=nm, tag="ut")

        def emit_qk(c0, c1, m, bias_t, out_t, nm):
            w = c1 - c0
            ps = ut_tile(nm)
            for t in range(2):
                nc.tensor.matmul(
                    ps[:, :w], wqk_sb[t][:, 128 * m:128 * m + 128], xT[t][:, c0:c1],
                    start=(t == 0), stop=(t == 1),
                )
            nc.vector.tensor_scalar_add(out_t[:, c0:c1], ps[:, :w], bias_t[:])

        def emit_v(i, nm):
            o, sz = KTILES[i]
            ps = ut_tile(nm)
            for t in range(2):
                nc.tensor.matmul(
                    ps[:sz, 0:GC], xT[t][:, o:o + sz], wv_sb[t][:],
                    start=(t == 0), stop=(t == 1),
                )
            nc.vector.tensor_copy(Vt[i][:sz, :], ps[:sz, 0:GC])

        def emit_scores(c0, w, i, half, off=0):
            # head h -> its own PSUM bank (concurrent row-packed matmuls
            # must not share a bank). No half alternation: the conservative
            # bbox dependency serializes scores<->exp anyway, so each head
            # uses its full 512-col bank and groups are 2x wider (fewer
            # chain links, fewer per-instr overheads).
            o, sz = KTILES[i]
            for h in range(4):
                base = 512 * h + off
                nc.tensor.matmul(
                    st[:sz, base:base + w],
                    KTt[32 * h:32 * h + 32, o:o + sz],
                    QT[32 * h:32 * h + 32, c0:c0 + w],
                    start=True, stop=True,
                    tile_position=(32 * h, 0),
                )

        def emit_exp(w, sz, half, P, nm, off=0):
            stv = st_h[:sz, :, off:off + w]
            if w == PW:
                nc.scalar.activation(P[:sz, :], st[:sz, :], Exp)
            else:
                pv = P[:sz, :].rearrange("p (h c) -> p h c", h=4)[:, :, 0:w]
                nc.scalar.activation(pv, stv, Exp)

        def emit_av(P, w, i, attnT_ps, start, stop):
            o, sz = KTILES[i]
            for h in range(4):
                nc.tensor.matmul(
                    attnT_ps[32 * h:32 * h + 32, :w],
                    Vt[i][:sz, 32 * h:32 * h + 32],
                    P[:sz, PW * h:PW * h + w],
                    start=start, stop=stop, skip_group_check=True,
                    tile_position=(0, 32 * h),
                )

        def emit_den(P, w, i, den_ps, start, stop):
            o, sz = KTILES[i]
            for h in range(4):
                nc.tensor.matmul(
                    den_ps[32 * h:32 * h + 32, 0:w],
                    ones_b[:sz, :],
                    P[:sz, PW * h:PW * h + w],
                    start=start, stop=stop, skip_group_check=True,
                    tile_position=(0, 32 * h),
                )

        def emit_outproj(attn_sb, c0, w, s4, nm):
            ssz = min(128, w - 128 * s4)
            off = 128 * s4
            yp = ut_tile(f"yp{nm}{s4}")
            nc.tensor.matmul(
                yp[:ssz, 0:256], attn_sb[:, off:off + ssz], wout_sb[:],
                start=True, stop=True,
            )
            ysb = sb.tile([128, 256], f32, name=f"ys{nm}{s4}", tag="ysb")
            nc.vector.tensor_copy(ysb[:ssz, :], yp[:ssz, 0:256])
            nc.sync.dma_start(y_d[c0 + off:c0 + off + ssz, :], ysb[:ssz, :])

        def emit_tail(c0, w, attnT_ps, den_ps, nm, spread=None):
            # den is broadcast across each head's 32 partitions (ones lhsT
            # is [sz, 32]), so normalization is reciprocal + multiply only.
            recip_f = sb.tile([128, PW], f32, name=f"rf{nm}", tag="recipf")
            nc.vector.reciprocal_approx_fast(recip_f[:, :w], den_ps[:, :w])
            attn_sb = sb.tile([128, PW], bf16, name=f"at{nm}", tag="attnsb")
            nc.vector.tensor_mul(attn_sb[:, :w], attnT_ps[:, :w], recip_f[:, :w])
            nsub = (w + 127) // 128
            for s4 in range(nsub):
                th = (lambda a=attn_sb, c0=c0, w=w, s4=s4, nm=nm:
                      emit_outproj(a, c0, w, s4, nm))
                if spread is None:
                    th()
                else:
                    spread.append(th)

        # ---- deferred production schedule ----
        # K chunk j covers score k-tiles 2j, 2j+1 -> keep 3-4 tiles ahead.
        # V k-tile j is consumed by group (0, j) -> produce at (0, j-2).
        # Q chunk c+1 is consumed from chunk c+1 -> produce mid-chunk c.
        pending = {}

        def defer(c, i, th):
            pending.setdefault((c, i), []).append(th)

        # K chunk j covers tokens [c0, c0+w); first score k-tile touching
        # it is floor(c0/128) — emit 2-3 k-tiles ahead.
        for j in range(1, NCH):
            c0j, wj = CHUNKS[j]
            ii = max(0, c0j // 128 - 3)
            defer(0, ii, lambda c0j=c0j, wj=wj, j=j: emit_qk(
                c0j, c0j + wj, 1, bk_sb, KTt, f"k{j}"))
        for j in range(2, NKT):
            defer(0, j - 2, lambda j=j: emit_v(j, f"v{j}"))
        for c in range(NCH - 1):
            defer(c, 6, lambda c=c: emit_qk(
                CHUNKS[c + 1][0], CHUNKS[c + 1][0] + CHUNKS[c + 1][1],
                0, bq_sb, QT, f"q{c+1}"))

        # ---- prologue ----
        emit_qk(0, 512, 0, bq_sb, QT, "q0")
        emit_qk(0, 512, 1, bk_sb, KTt, "k0")
        emit_v(0, "v0")
        emit_v(1, "v1")

        # ---- main pipeline over NCH chunks x NKT ktiles ----
        groups = [(c, i) for c in range(3) for i in range(NKT)]
        emit_scores(0, CHUNKS[0][1], 0, 0)
        attnT_ps = den_ps = None
        tail_thunk = None
        avden_prev = None
        op_q = []
        for g, (c, i) in enumerate(groups):
            cw0, cww = CHUNKS[c]
            if i == 0:
                attnT_ps = avps.tile([128, PW], f32, name=f"attnT{c}", tag="attnT",
                                     padded_shape=[128, 512])
                den_ps = dnps.tile([128, PW], f32, name=f"den{c}", tag="den",
                                   padded_shape=[128, 512])
            o, sz = KTILES[i]
            P = psb.tile([128, 4 * PW], bf16, name=f"P{c}_{i}", tag="P")
            # Without half alternation scores(g+1) overwrite the slot
            # exp(g) reads, so exp MUST be emitted first (the tracker pairs
            # each read with the last prior write). The resulting chain
            # exp(g) -> scores(g+1) -> exp(g+1) is the same serialization
            # the bbox edges forced anyway; wider groups amortize it.
            emit_exp(cww, sz, g % 2, P, f"e{c}_{i}")
            if g + 1 < len(groups):
                nc2, ni = groups[g + 1]
                emit_scores(CHUNKS[nc2][0], CHUNKS[nc2][1], ni, (g + 1) % 2)
            # tail of chunk c-1 precedes this chunk's first AV (avps/dnps
            # have one buf; the pool WAR edge needs mul already emitted)
            if tail_thunk is not None and i == 1:
                tail_thunk()
                tail_thunk = None
            if op_q and i >= 2:
                op_q.pop(0)()
            # deferred by one group so they never gate the next scores:
            # AV/den of group g-1 run on PE while ACT exps group g
            if avden_prev is not None:
                avden_prev()
                avden_prev = None
            avden_prev = (lambda P=P, w=cww, i=i, a=attnT_ps, d=den_ps:
                          (emit_av(P, w, i, a, start=(i == 0), stop=(i == NKT - 1)),
                           emit_den(P, w, i, d, start=(i == 0), stop=(i == NKT - 1))))
            for th in pending.get((c, i), ()):
                th()
            if i == NKT - 1:
                tail_thunk = (lambda c0=cw0, w=cww, a=attnT_ps, d=den_ps, c=c:
                              emit_tail(c0, w, a, d, f"t{c}", spread=op_q))
        avden_prev()
        avden_prev = None

        # ---- rump chunk (q=1536:1568, w=32): all 13 k-tiles packed into
        # ONE score/exp group (k-tile j at cols 512h+32j), one chain link
        # (~4us) instead of 13 serialized tiny groups (~11us) ----
        for j in range(NKT):
            emit_scores(1536, 32, j, 0, off=32 * j)
        PR = psb.tile([128, 4 * PW], bf16, name="PR", tag="P")
        # two exp halves: AV/den of k-tiles 0-5 run under exp of 6-12
        stvA = st_h[:, :, 0:192]
        pvA = PR[:, :].rearrange("p (h c) -> p h c", h=4)[:, :, 0:192]
        nc.scalar.activation(pvA, stvA, Exp)
        stvB = st_h[:, :, 192:32 * NKT]
        pvB = PR[:, :].rearrange("p (h c) -> p h c", h=4)[:, :, 192:32 * NKT]
        nc.scalar.activation(pvB, stvB, Exp)
        if tail_thunk is not None:
            tail_thunk()
            tail_thunk = None
        for th in op_q:
            th()
        op_q = []
        attnT_ps = avps.tile([128, PW], f32, name="attnTr", tag="attnT",
                             padded_shape=[128, 512])
        den_ps = dnps.tile([128, PW], f32, name="denr", tag="den",
                           padded_shape=[128, 512])
        for j in range(NKT):
            o, sz = KTILES[j]
            for h in range(4):
                nc.tensor.matmul(
                    attnT_ps[32 * h:32 * h + 32, 0:32],
                    Vt[j][:sz, 32 * h:32 * h + 32],
                    PR[:sz, 512 * h + 32 * j:512 * h + 32 * j + 32],
                    start=(j == 0), stop=(j == NKT - 1), skip_group_check=True,
                    tile_position=(0, 32 * h),
                )
            for h in range(4):
                nc.tensor.matmul(
                    den_ps[32 * h:32 * h + 32, 0:32],
                    ones_b[:sz, :],
                    PR[:sz, 512 * h + 32 * j:512 * h + 32 * j + 32],
                    start=(j == 0), stop=(j == NKT - 1), skip_group_check=True,
                    tile_position=(0, 32 * h),
                )
        emit_tail(1536, 32, attnT_ps, den_ps, "tr")

        ctx.close()

    nc.compile()
    return nc


def prepare_in_maps(x1, x2, pos_emb, w_qkv, b_qkv, w_out, b_out):
    import ml_dtypes

    bf16 = ml_dtypes.bfloat16
    x1 = np.asarray(x1, dtype=np.float32)
    x2 = np.asarray(x2, dtype=np.float32)
    pos = np.asarray(pos_emb, dtype=np.float32).reshape(C)
    w_qkv = np.asarray(w_qkv, dtype=np.float32)
    b_qkv = np.asarray(b_qkv, dtype=np.float32)
    w_out = np.asarray(w_out, dtype=np.float32)
    b_out = np.asarray(b_out, dtype=np.float32)

    scale = 1.0 / np.sqrt(np.float32(DH))
    b_eff = b_qkv + w_qkv @ pos
    wq = w_qkv[0:C] * scale
    bq = b_eff[0:C] * scale
    wk = w_qkv[C:2 * C]
    bk = b_eff[C:2 * C]
    wv = w_qkv[2 * C:3 * C]
    bv = b_eff[2 * C:3 * C]

    in_maps = []
    for core in range(8):
        b = core // 2
        g = core % 2
        gsl = slice(GC * g, GC * (g + 1))
        wqkT = np.concatenate([wq[gsl], wk[gsl]], axis=0).T.copy()     # [C, 256]
        wvT = wv[gsl].T.copy()                                         # [C, GC]
        woutT = w_out[:, gsl].T.copy()                                 # [GC, 256]
        bqk = np.stack([bq[gsl], bk[gsl]])[:, :, None].copy()          # [2, 128, 1]
        in_maps.append({
            "x1b": np.ascontiguousarray(x1[b].reshape(C, N)).astype(bf16),
            "x2b": np.ascontiguousarray(x2[b].reshape(C, N)).astype(bf16),
            "wqkT": np.ascontiguousarray(wqkT).astype(bf16),
            "wvT": np.ascontiguousarray(wvT).astype(bf16),
            "woutT": np.ascontiguousarray(woutT).astype(bf16),
            "bqk": np.ascontiguousarray(bqk),
        })
    # out1+out2 folds two tokens, each carrying b_out and the V-bias term
    y_const = 2.0 * (b_out + w_out @ bv)  # [C]
    return in_maps, y_const


def get_nc(repeat=1):
    key = repeat
    if key not in _cache:
        _cache[key] = _build_nc(repeat)
    return _cache[key]


def assemble(per_core_y, y_const):
    out = np.empty((B, C, H, W), dtype=np.float32)
    for b in range(B):
        yb = per_core_y[2 * b] + per_core_y[2 * b + 1]                 # [S, C]
        yf = yb[:N] + yb[N:] + y_const[None, :]                        # [N, C]
        out[b] = yf.T.reshape(C, H, W)
    return out


def kernel(x1, x2, pos_emb, w_qkv, b_qkv, w_out, b_out):
    global LAST_RESULTS, LAST_IN_MAPS
    from concourse.bass_utils import run_bass_kernel_spmd

    in_maps, y_const = prepare_in_maps(x1, x2, pos_emb, w_qkv, b_qkv, w_out, b_out)
    LAST_IN_MAPS = in_maps
    nc = get_nc()
    res = run_bass_kernel_spmd(nc, in_maps, core_ids=list(range(8)))
    LAST_RESULTS = res
    return assemble([res.results[c]["y"] for c in range(8)], y_const)
